# revision 34
# baseline (speedup 1.0000x reference)
"""Trainium2 Bass kernel for nn_AutoCorrelation (multi-head attention with a
distance decay bias), SPMD across 8 NeuronCores.

Sharding: core = (batch b, head-group hg) with b in 0..3, hg in 0..1.
Each core computes, for its batch and its 8 heads: QKV projections
(column-sharded weights), distance-banded attention (the -0.1*|i-j| bias makes
weights beyond |i-j|~96 numerically zero), and a row-sharded output
projection. The host sums the two half partial outputs per batch and adds the
effective output bias.

Math notes:
 - bk drops out entirely (softmax row-shift invariance); bv is folded into the
   host-side output bias: bo_eff = bo + Wo @ bv.
 - scores are built transposed St[k, q]; a ones-column appended to V yields the
   softmax denominators in the same matmul (PSUM row 64).
 - the bias exp(-0.1|k-q|) is a Toeplitz multiply from a precomputed master.

Structure (v2):
 - warmup matmuls at t=0 keep the PE HAM un-throttled while input DMA streams;
   weight DMAs issue from gpsimd in parallel with x loads on sync.
 - Phase B runs as 4 q-tile passes; per pass all 8 heads are processed as 4
   pairs with concurrent K=64 score matmuls (tile_position row groups), one
   merged exp ACTIVATE per pair, and the output projection + out DMA run
   inside the pass so HBM writes spread across the kernel.
"""

import math
from contextlib import ExitStack

import numpy as np
import ml_dtypes

BF16 = ml_dtypes.bfloat16

N_CORES = 8


class Cfg:
    def __init__(self, L=2048, C=1024, NHL=8, DK=64, W=96):
        self.L, self.C, self.NHL, self.DK, self.W = L, C, NHL, DK, W
        self.DL = NHL * DK               # local head dims
        self.SPAN = 128 + 2 * W          # k-chunk q-span
        self.KC = L // 128               # k chunks
        self.NQT = L // 512              # q tiles (512)
        self.CC = C // 128               # contraction chunks
        self.LT = L // 512               # l tiles
        self.HP = NHL // 2               # head pairs
        self.VW = NHL * 65               # padded V width
        self.EBW = self.SPAN + 576       # EB master width (seb range + span)
        assert self.SPAN <= self.L

    def qs_of(self, kc):
        return min(max(128 * kc - self.W, 0), self.L - self.SPAN)

    def pieces_of(self, qt):
        """(kc, q0, N) pieces whose span intersects q-tile qt."""
        lo, hi = 512 * qt, 512 * qt + 512
        out = []
        for kc in range(self.KC):
            qs = self.qs_of(kc)
            q0, q1 = max(qs, lo), min(qs + self.SPAN, hi)
            if q1 > q0:
                out.append((kc, q0, q1 - q0))
        return out


FULL = Cfg(W=80)


def build_program(cfg=FULL, debug=False):
    import concourse.bass as bass
    import concourse.tile as tile
    from concourse import bacc, mybir

    f32 = mybir.dt.float32
    bf16 = mybir.dt.bfloat16
    AF = mybir.ActivationFunctionType

    L, C, NHL, DL = cfg.L, cfg.C, cfg.NHL, cfg.DL
    KC, NQT, CC, LT, HP, VW = cfg.KC, cfg.NQT, cfg.CC, cfg.LT, cfg.HP, cfg.VW

    nc = bacc.Bacc("TRN2", target_bir_lowering=False, debug=debug,
                   num_devices=N_CORES)

    xq = nc.dram_tensor("xq", [C, L], bf16, kind="ExternalInput").ap()
    xk = nc.dram_tensor("xk", [C, L], bf16, kind="ExternalInput").ap()
    xv = nc.dram_tensor("xv", [C, L], bf16, kind="ExternalInput").ap()
    wq = nc.dram_tensor("wq", [C, DL], bf16, kind="ExternalInput").ap()
    wk = nc.dram_tensor("wk", [C, DL], bf16, kind="ExternalInput").ap()
    wv = nc.dram_tensor("wv", [C, DL], bf16, kind="ExternalInput").ap()
    wo = nc.dram_tensor("wo", [DL, C], bf16, kind="ExternalInput").ap()
    bqd = nc.dram_tensor("bq", [DL, 1], f32, kind="ExternalInput").ap()
    ebd = nc.dram_tensor("eb", [128, cfg.EBW], bf16, kind="ExternalInput").ap()
    idnd = nc.dram_tensor("idn", [128, 128], bf16, kind="ExternalInput").ap()
    out = nc.dram_tensor("out", [L, C], bf16, kind="ExternalOutput").ap()

    with tile.TileContext(nc) as tc, ExitStack() as ctx:
        const = ctx.enter_context(tc.tile_pool(name="const", bufs=1))
        big = ctx.enter_context(tc.tile_pool(name="big", bufs=1))
        xs = ctx.enter_context(tc.tile_pool(name="xs", bufs=3))
        ets = ctx.enter_context(tc.tile_pool(name="ets", bufs=4))
        rbp = ctx.enter_context(tc.tile_pool(name="rbp", bufs=2))
        ostage = ctx.enter_context(tc.tile_pool(name="ostage", bufs=2))
        psum = ctx.enter_context(tc.tile_pool(name="psum", bufs=1, space="PSUM"))

        # ---- PE warmup: keep HAM un-throttled while input DMA streams ----
        dummy = const.tile([128, 512], bf16)
        nc.vector.memset(dummy[:], 0.0)
        # dummy exp pulls the ACT table load off the critical path
        dummy_act = const.tile([1, 8], f32)
        nc.scalar.activation(dummy_act[:], dummy[0:1, 0:8], AF.Exp, scale=0.125)
        warm_ps = psum.tile([128, 1024], f32, tag="pa", bufs=2, name="warm")
        for i in range(28):
            nc.tensor.matmul(warm_ps[:, 0:512], lhsT=dummy[:, 0:128],
                             rhs=dummy[:], start=True, stop=True)
        warm_sink = const.tile([128, 512], bf16)
        nc.vector.tensor_copy(warm_sink[:], warm_ps[:, 0:512])

        # ---- resident constants (weight DMAs on gpsimd, x loads on sync) ----
        wq_sb = const.tile([128, CC * DL], bf16)
        wk_sb = const.tile([128, CC * DL], bf16)
        wv_sb = const.tile([128, CC * DL], bf16)
        wo_sb = const.tile([128, HP * C], bf16)
        bq_sb = const.tile([128, HP], f32)
        for hp in range(HP):
            nc.gpsimd.dma_start(bq_sb[:, hp:hp + 1], bqd[hp * 128:(hp + 1) * 128, :])
        # two 128-row chunks per descriptor: halves the ~600ns-per-dma_start
        # issue cost that was starving the early projection matmuls
        def load_w2(dst_sb, src):
            for c2 in range(CC // 2):
                nc.gpsimd.dma_start(
                    dst_sb[:, c2 * 2 * DL:(c2 + 1) * 2 * DL].rearrange(
                        "p (r w) -> p r w", r=2),
                    src[c2 * 256:(c2 + 1) * 256, :].rearrange(
                        "(r p) w -> p r w", p=128))

        load_w2(wq_sb, wq)
        load_w2(wk_sb, wk)
        load_w2(wv_sb, wv)
        eb_sb = const.tile([128, cfg.EBW], bf16)
        nc.gpsimd.dma_start(eb_sb[:], ebd[:])
        idn_sb = const.tile([128, 128], bf16)
        nc.gpsimd.dma_start(idn_sb[:], idnd[:])
        for hp in range(HP):
            nc.gpsimd.dma_start(wo_sb[:, hp * C:(hp + 1) * C], wo[hp * 128:(hp + 1) * 128, :])

        # ---- resident activations ----
        qt_sb = [big.tile([128, L], bf16, name=f"qt{hp}") for hp in range(HP)]
        kt_sb = [big.tile([128, L], bf16, name=f"kt{hp}") for hp in range(HP)]
        vb_sb = big.tile([128, KC * VW], bf16)
        ots_sb = [big.tile([128, L], bf16, name=f"ots{hp}") for hp in range(HP)]

        # ones columns of the [V_h | ones] layout, written once on DVE
        for kcg in range(KC):
            vbk = vb_sb[:, kcg * VW:(kcg + 1) * VW].rearrange(
                "p (h w) -> p h w", w=65)
            nc.vector.memset(vbk[:, :, 64:65], 1.0)

        # ================= Phase A: projections =================
        # Alternate projection PSUM groups across both ring tags ("pa"/"sc")
        # for an effective 4-deep pipeline: the drain of group i no longer
        # gates the matmuls of group i+2.
        alt = [0]

        def proj_ps(shape, name):
            alt[0] += 1
            tag = "pa" if alt[0] % 2 else "sc"
            return psum.tile(shape, f32, tag=tag, bufs=2, name=name)

        for lt in range(LT):
            for which, xdram in (("q", xq), ("k", xk), ("v", xv)):
                x_sb = xs.tile([128, CC * 512], bf16, tag="xs",
                               name=f"x_{which}{lt}")
                for c2 in range(CC // 2):
                    nc.sync.dma_start(
                        x_sb[:, c2 * 1024:(c2 + 1) * 1024].rearrange(
                            "p (r w) -> p r w", r=2),
                        xdram[c2 * 256:(c2 + 1) * 256,
                              lt * 512:(lt + 1) * 512].rearrange(
                            "(r p) w -> p r w", p=128))
                if which in ("q", "k"):
                    w_sb = wq_sb if which == "q" else wk_sb
                    t_sb = qt_sb if which == "q" else kt_sb
                    for hp in range(HP):
                        ps = proj_ps([128, 512], f"psp_{which}{lt}_{hp}")
                        for c in range(CC):
                            nc.tensor.matmul(
                                ps[:],
                                lhsT=w_sb[:, c * DL + hp * 128: c * DL + hp * 128 + 128],
                                rhs=x_sb[:, c * 512:(c + 1) * 512],
                                start=(c == 0), stop=(c == CC - 1))
                        dst = t_sb[hp][:, lt * 512:(lt + 1) * 512]
                        if which == "q":
                            nc.scalar.activation(dst, ps[:], AF.Identity,
                                                 bias=bq_sb[:, hp:hp + 1], scale=1.0)
                        else:
                            nc.vector.tensor_copy(dst, ps[:])
                else:
                    for sub in range(4):
                        kcg = lt * 4 + sub
                        ps = proj_ps([128, DL], f"psp_v{kcg}")
                        for c in range(CC):
                            lhsT = x_sb[:, c * 512 + sub * 128: c * 512 + sub * 128 + 128]
                            nc.tensor.matmul(
                                ps[:], lhsT=lhsT,
                                rhs=wv_sb[:, c * DL:(c + 1) * DL],
                                start=(c == 0), stop=(c == CC - 1))
                        vbk = vb_sb[:, kcg * VW:(kcg + 1) * VW].rearrange(
                            "p (h w) -> p h w", w=65)
                        nc.vector.tensor_copy(
                            vbk[:, :, 0:64],
                            ps.rearrange("p (h w) -> p h w", w=64))

        # ================= Phase B: banded attention, q-tile passes =========
        # Flat software pipeline over all (qt, head-pair, kc-piece) units with
        # one-piece lookahead crossing pair and qt boundaries.  PSUM: "sc"
        # ring 2x2 banks (scores + outproj pf), "pa" ring 2x2 banks (Phase A
        # proj + po accumulators) = 8 banks.  The distance bias -0.8|k-q| is
        # added into the score PSUM by an identity matmul, so exp(0.125*x)
        # yields the biased weights directly (no separate DVE multiply).

        def outproj_qc(qc):
            st = ostage.tile([128, C], bf16, tag="fo", name=f"fo{qc}")
            pf = psum.tile([128, 1024], f32, tag="sc", bufs=2, name=f"pf{qc}")
            for mi, mo in enumerate((0, 512)):
                for hp2 in range(HP):
                    nc.tensor.matmul(
                        pf[:, mo:mo + 512],
                        lhsT=ots_sb[hp2][:, qc * 128:(qc + 1) * 128],
                        rhs=wo_sb[:, hp2 * C + mo: hp2 * C + mo + 512],
                        start=(hp2 == 0), stop=(hp2 == HP - 1))
            # split the drain across both engines so the pf ring slot frees
            # in half the time
            nc.scalar.copy(st[:, 0:512], pf[:, 0:512])
            nc.vector.tensor_copy(st[:, 512:1024], pf[:, 512:1024])
            nc.gpsimd.dma_start(out[qc * 128:(qc + 1) * 128, :], st[:])

        units = []
        qt_start = {}
        for qt in range(NQT):
            pieces = cfg.pieces_of(qt)
            # greedy-pack consecutive kc-pieces into groups of total width
            # <=512 so each group needs one PSUM bank per head and one exp
            groups = []
            for kc, q0, n in pieces:
                if groups and groups[-1][-1][3] + groups[-1][-1][2] + n <= 512:
                    off = groups[-1][-1][3] + groups[-1][-1][2]
                    groups[-1].append((kc, q0, n, off))
                else:
                    groups.append([(kc, q0, n, 0)])
            qt_start[qt] = len(units)
            for hp in range(HP):
                for idx, subs in enumerate(groups):
                    units.append(dict(
                        qt=qt, hp=hp, subs=subs,
                        first=(idx == 0), last=(idx == len(groups) - 1)))
        # outproj(qt) is delayed into qt+1's unit stream (one qc per ~2
        # units) so the normalization chain overlaps flowing attention work
        # and the outproj matmuls act as PE filler
        after_unit = {}
        for qt in range(NQT):
            for j in range(4):
                qc = 4 * qt + j
                if qt + 1 < NQT:
                    key = qt_start[qt + 1] + 4 * j + 1
                else:
                    key = len(units) - 1
                after_unit.setdefault(key, []).append(qc)

        state = {}

        def scores(u):
            v = units[u]
            sc = psum.tile([128, 1024], f32, tag="sc", bufs=2, name=f"sc{u}")
            state[u] = sc
            for si, (kc, q0, n, off) in enumerate(v["subs"]):
                for hi in range(2):
                    nc.tensor.matmul(
                        sc[:, hi * 512 + off: hi * 512 + off + n],
                        lhsT=kt_sb[v["hp"]][hi * 64:(hi + 1) * 64,
                                            kc * 128:(kc + 1) * 128],
                        rhs=qt_sb[v["hp"]][hi * 64:(hi + 1) * 64, q0: q0 + n],
                        start=(si == 0), stop=False)
            last = len(v["subs"]) - 1
            for si, (kc, q0, n, off) in enumerate(v["subs"]):
                seb = q0 - 128 * kc + 512
                for hi in range(2):
                    nc.tensor.matmul(
                        sc[:, hi * 512 + off: hi * 512 + off + n],
                        lhsT=idn_sb[:],
                        rhs=eb_sb[:, seb:seb + n],
                        start=False, stop=(si == last))

        def finish(u):
            v = units[u]
            qt, hp = v["qt"], v["hp"]
            sc = state.pop(u)
            if v["first"]:
                state[("po", qt, hp)] = psum.tile(
                    [65, 1024], f32, tag="pa", bufs=2, name=f"po{qt}_{hp}")
            po = state[("po", qt, hp)]
            wg = v["subs"][-1][3] + v["subs"][-1][2]  # group width
            et = ets.tile([128, 1024], bf16, tag="et", name=f"et{u}")
            et2 = et.rearrange("p (r w) -> p r w", r=2)
            sc3 = sc.rearrange("p (r w) -> p r w", r=2)
            nc.scalar.activation(et2[:, :, 0:wg], sc3[:, :, 0:wg],
                                 AF.Exp, scale=0.125)
            last = len(v["subs"]) - 1
            for si, (kc, q0, n, off) in enumerate(v["subs"]):
                qoff = q0 - 512 * qt
                for hi in range(2):
                    h = 2 * hp + hi
                    vsl = vb_sb[:, kc * VW + h * 65: kc * VW + h * 65 + 65]
                    nc.tensor.matmul(
                        po[0:65, hi * 512 + qoff: hi * 512 + qoff + n],
                        lhsT=vsl,
                        rhs=et2[:, hi, off:off + n],
                        start=(v["first"] and si == 0),
                        stop=(v["last"] and si == last))
            if not v["last"]:
                return
            # normalize: denominators sit in po row 64 (both heads); stage to
            # partition 0 first (the custom-DVE recip can't cross 32-strips)
            po = state.pop(("po", qt, hp))
            s_st = rbp.tile([1, 1024], f32, tag="ss", name=f"ss{qt}_{hp}")
            if hp % 2 == 0:
                nc.scalar.copy(s_st[:], po[64:65, :])
            else:
                nc.vector.tensor_copy(s_st[:], po[64:65, :])
            r_f = rbp.tile([1, 1024], f32, tag="rf", name=f"rf{qt}_{hp}")
            nc.vector.reciprocal_approx_fast(r_f[:], s_st[:])
            r_b = rbp.tile([1, 1024], bf16, tag="rb", name=f"rb{qt}_{hp}")
            nc.vector.tensor_copy(r_b[:], r_f[:])
            rbb = rbp.tile([64, 1024], bf16, tag="rbb", name=f"rbb{qt}_{hp}")
            nc.gpsimd.partition_broadcast(rbb[:], r_b[:])
            for hi in range(2):
                sl = (slice(hi * 64, (hi + 1) * 64),
                      slice(qt * 512, (qt + 1) * 512))
                nc.vector.tensor_mul(
                    ots_sb[hp][sl],
                    po[0:64, hi * 512: hi * 512 + 512],
                    rbb[:, hi * 512: hi * 512 + 512])

        scores(0)
        for u in range(1, len(units)):
            scores(u)
            finish(u - 1)
            for qc in after_unit.get(u - 1, ()):
                outproj_qc(qc)
        finish(len(units) - 1)
        for qc in after_unit.get(len(units) - 1, ()):
            outproj_qc(qc)

    nc.compile()
    return nc


def host_inputs(inputs, cfg=FULL):
    """Build the 8 per-core input maps + the host-side combine constant."""
    L, C, DL, NHL = cfg.L, cfg.C, cfg.DL, cfg.NHL
    q = np.asarray(inputs["queries"], np.float32)
    k = np.asarray(inputs["keys"], np.float32)
    v = np.asarray(inputs["values"], np.float32)
    Wq = np.asarray(inputs["Wq"], np.float32)
    Wk = np.asarray(inputs["Wk"], np.float32)
    Wv = np.asarray(inputs["Wv"], np.float32)
    Wo = np.asarray(inputs["Wo"], np.float32)
    bq = np.asarray(inputs["bq"], np.float32)
    bv = np.asarray(inputs["bv"], np.float32)
    bo = np.asarray(inputs["bo"], np.float32)
    B = q.shape[0]

    bo_eff = (bo.astype(np.float64) + Wo.astype(np.float64) @ bv.astype(np.float64)
              ).astype(np.float32)

    p = np.arange(128, dtype=np.float64)[:, None]
    c = np.arange(cfg.EBW, dtype=np.float64)[None, :]
    # additive log-bias, pre-divided by the 0.125 softmax scale:
    # exp(0.125*(s + eb)) = exp(s/8 - 0.1|k-q|)
    eb = (-0.8 * np.abs(p - c + 512)).astype(BF16)
    idn = np.eye(128, dtype=BF16)

    xT = {}
    for b in range(B):
        xT[b] = (np.ascontiguousarray(q[b].T).astype(BF16),
                 np.ascontiguousarray(k[b].T).astype(BF16),
                 np.ascontiguousarray(v[b].T).astype(BF16))

    in_maps = []
    for core in range(N_CORES):
        b, hg = core // 2, core % 2
        sl = slice(hg * DL, (hg + 1) * DL)
        in_maps.append({
            "xq": xT[b][0], "xk": xT[b][1], "xv": xT[b][2],
            "wq": np.ascontiguousarray(Wq.T[:, sl]).astype(BF16),
            "wk": np.ascontiguousarray(Wk.T[:, sl]).astype(BF16),
            "wv": np.ascontiguousarray(Wv.T[:, sl]).astype(BF16),
            "wo": np.ascontiguousarray(Wo.T[sl, :]).astype(BF16),
            "bq": np.ascontiguousarray(bq[sl][:, None]),
            "eb": eb, "idn": idn,
        })
    return in_maps, bo_eff


_CACHED = {}


def _wait_devices_healthy(timeout_s=420):
    import time
    import jax
    import jax.numpy as jnp
    t0 = time.time()
    last = None
    while time.time() - t0 < timeout_s:
        try:
            for d in jax.devices():
                x = jax.device_put(np.ones((8, 8), np.float32), d)
                jnp.sum(x).block_until_ready()
            return
        except Exception as e:  # wedged worker recycles within a few minutes
            last = e
            time.sleep(15)
    raise RuntimeError(f"NeuronCores unhealthy after {timeout_s}s: {last}")


def kernel(**inputs):
    from concourse.bass_utils import run_bass_kernel_spmd

    cfg = FULL
    if "nc" not in _CACHED:
        _CACHED["nc"] = build_program(cfg)
    nc = _CACHED["nc"]

    in_maps, bo_eff = host_inputs(inputs, cfg)
    _wait_devices_healthy()
    try:
        res = run_bass_kernel_spmd(nc, in_maps, core_ids=list(range(N_CORES)))
    except Exception:
        _wait_devices_healthy()
        res = run_bass_kernel_spmd(nc, in_maps, core_ids=list(range(N_CORES)))
    B = np.asarray(inputs["queries"]).shape[0]
    out = np.zeros((B, cfg.L, cfg.C), np.float32)
    for b in range(B):
        out[b] = (res.results[2 * b]["out"].astype(np.float32)
                  + res.results[2 * b + 1]["out"].astype(np.float32)
                  + bo_eff[None, :])
    return out


# revision 36
# speedup vs baseline: 1.0285x; 1.0285x over previous
"""Trainium2 Bass kernel for nn_AutoCorrelation (multi-head attention with a
distance decay bias), SPMD across 8 NeuronCores.

Sharding: core = (batch b, head-group hg) with b in 0..3, hg in 0..1.
Each core computes, for its batch and its 8 heads: QKV projections
(column-sharded weights), distance-banded attention (the -0.1*|i-j| bias makes
weights beyond |i-j|~96 numerically zero), and a row-sharded output
projection. The host sums the two half partial outputs per batch and adds the
effective output bias.

Math notes:
 - bk drops out entirely (softmax row-shift invariance); bv is folded into the
   host-side output bias: bo_eff = bo + Wo @ bv.
 - scores are built transposed St[k, q]; a ones-column appended to V yields the
   softmax denominators in the same matmul (PSUM row 64).
 - the bias exp(-0.1|k-q|) is a Toeplitz multiply from a precomputed master.

Structure (v2):
 - warmup matmuls at t=0 keep the PE HAM un-throttled while input DMA streams;
   weight DMAs issue from gpsimd in parallel with x loads on sync.
 - Phase B runs as 4 q-tile passes; per pass all 8 heads are processed as 4
   pairs with concurrent K=64 score matmuls (tile_position row groups), one
   merged exp ACTIVATE per pair, and the output projection + out DMA run
   inside the pass so HBM writes spread across the kernel.
"""

import math
from contextlib import ExitStack

import numpy as np
import ml_dtypes

BF16 = ml_dtypes.bfloat16

N_CORES = 8


class Cfg:
    def __init__(self, L=2048, C=1024, NHL=8, DK=64, W=96):
        self.L, self.C, self.NHL, self.DK, self.W = L, C, NHL, DK, W
        self.DL = NHL * DK               # local head dims
        self.SPAN = 128 + 2 * W          # k-chunk q-span
        self.KC = L // 128               # k chunks
        self.NQT = L // 512              # q tiles (512)
        self.CC = C // 128               # contraction chunks
        self.LT = L // 512               # l tiles
        self.HP = NHL // 2               # head pairs
        self.VW = NHL * 65               # padded V width
        self.EBW = self.SPAN + 576       # EB master width (seb range + span)
        assert self.SPAN <= self.L

    def qs_of(self, kc):
        return min(max(128 * kc - self.W, 0), self.L - self.SPAN)

    def pieces_of(self, qt):
        """(kc, q0, N) pieces whose span intersects q-tile qt."""
        lo, hi = 512 * qt, 512 * qt + 512
        out = []
        for kc in range(self.KC):
            qs = self.qs_of(kc)
            q0, q1 = max(qs, lo), min(qs + self.SPAN, hi)
            if q1 > q0:
                out.append((kc, q0, q1 - q0))
        return out


FULL = Cfg(W=80)


def build_program(cfg=FULL, debug=False):
    import concourse.bass as bass
    import concourse.tile as tile
    from concourse import bacc, mybir

    f32 = mybir.dt.float32
    bf16 = mybir.dt.bfloat16
    AF = mybir.ActivationFunctionType

    L, C, NHL, DL = cfg.L, cfg.C, cfg.NHL, cfg.DL
    KC, NQT, CC, LT, HP, VW = cfg.KC, cfg.NQT, cfg.CC, cfg.LT, cfg.HP, cfg.VW

    nc = bacc.Bacc("TRN2", target_bir_lowering=False, debug=debug,
                   num_devices=N_CORES)

    xq = nc.dram_tensor("xq", [C, L], bf16, kind="ExternalInput").ap()
    xk = nc.dram_tensor("xk", [C, L], bf16, kind="ExternalInput").ap()
    xv = nc.dram_tensor("xv", [C, L], bf16, kind="ExternalInput").ap()
    wq = nc.dram_tensor("wq", [C, DL], bf16, kind="ExternalInput").ap()
    wk = nc.dram_tensor("wk", [C, DL], bf16, kind="ExternalInput").ap()
    wv = nc.dram_tensor("wv", [C, DL], bf16, kind="ExternalInput").ap()
    wo = nc.dram_tensor("wo", [DL, C], bf16, kind="ExternalInput").ap()
    bqd = nc.dram_tensor("bq", [DL, 1], f32, kind="ExternalInput").ap()
    ebd = nc.dram_tensor("eb", [128, cfg.EBW], bf16, kind="ExternalInput").ap()
    idnd = nc.dram_tensor("idn", [128, 128], bf16, kind="ExternalInput").ap()
    out = nc.dram_tensor("out", [L, C], bf16, kind="ExternalOutput").ap()

    with tile.TileContext(nc) as tc, ExitStack() as ctx:
        const = ctx.enter_context(tc.tile_pool(name="const", bufs=1))
        big = ctx.enter_context(tc.tile_pool(name="big", bufs=1))
        xs = ctx.enter_context(tc.tile_pool(name="xs", bufs=3))
        ets = ctx.enter_context(tc.tile_pool(name="ets", bufs=4))
        rbp = ctx.enter_context(tc.tile_pool(name="rbp", bufs=2))
        ostage = ctx.enter_context(tc.tile_pool(name="ostage", bufs=2))
        psum = ctx.enter_context(tc.tile_pool(name="psum", bufs=1, space="PSUM"))

        # ---- PE warmup: keep HAM un-throttled while input DMA streams ----
        dummy = const.tile([128, 512], bf16)
        nc.vector.memset(dummy[:], 0.0)
        # dummy exp pulls the ACT table load off the critical path
        dummy_act = const.tile([1, 8], f32)
        nc.scalar.activation(dummy_act[:], dummy[0:1, 0:8], AF.Exp, scale=0.125)
        warm_ps = psum.tile([128, 1024], f32, tag="pa", bufs=2, name="warm")
        for i in range(28):
            nc.tensor.matmul(warm_ps[:, 0:512], lhsT=dummy[:, 0:128],
                             rhs=dummy[:], start=True, stop=True)
        warm_sink = const.tile([128, 512], bf16)
        nc.vector.tensor_copy(warm_sink[:], warm_ps[:, 0:512])

        # ---- resident constants (weight DMAs on gpsimd, x loads on sync) ----
        wq_sb = const.tile([128, CC * DL], bf16)
        wk_sb = const.tile([128, CC * DL], bf16)
        wv_sb = const.tile([128, CC * DL], bf16)
        wo_sb = const.tile([128, HP * C], bf16)
        bq_sb = const.tile([128, HP], f32)
        for hp in range(HP):
            nc.gpsimd.dma_start(bq_sb[:, hp:hp + 1], bqd[hp * 128:(hp + 1) * 128, :])
        # two 128-row chunks per descriptor: halves the ~600ns-per-dma_start
        # issue cost that was starving the early projection matmuls
        def load_w2(dst_sb, src):
            for c2 in range(CC // 2):
                nc.gpsimd.dma_start(
                    dst_sb[:, c2 * 2 * DL:(c2 + 1) * 2 * DL].rearrange(
                        "p (r w) -> p r w", r=2),
                    src[c2 * 256:(c2 + 1) * 256, :].rearrange(
                        "(r p) w -> p r w", p=128))

        load_w2(wq_sb, wq)
        load_w2(wk_sb, wk)
        load_w2(wv_sb, wv)
        eb_sb = const.tile([128, cfg.EBW], bf16)
        nc.gpsimd.dma_start(eb_sb[:], ebd[:])
        idn_sb = const.tile([128, 128], bf16)
        nc.gpsimd.dma_start(idn_sb[:], idnd[:])
        for hp in range(HP):
            nc.gpsimd.dma_start(wo_sb[:, hp * C:(hp + 1) * C], wo[hp * 128:(hp + 1) * 128, :])

        # ---- resident activations ----
        qt_sb = [big.tile([128, L], bf16, name=f"qt{hp}") for hp in range(HP)]
        kt_sb = [big.tile([128, L], bf16, name=f"kt{hp}") for hp in range(HP)]
        vb_sb = big.tile([128, KC * VW], bf16)
        ots_sb = [big.tile([128, L], bf16, name=f"ots{hp}") for hp in range(HP)]

        # ones columns of the [V_h | ones] layout, written once on DVE
        for kcg in range(KC):
            vbk = vb_sb[:, kcg * VW:(kcg + 1) * VW].rearrange(
                "p (h w) -> p h w", w=65)
            nc.vector.memset(vbk[:, :, 64:65], 1.0)

        # ================= Phase A: projections =================
        # Alternate projection PSUM groups across both ring tags ("pa"/"sc")
        # for an effective 4-deep pipeline: the drain of group i no longer
        # gates the matmuls of group i+2.
        alt = [0]

        def proj_ps(shape, name):
            alt[0] += 1
            tag = "pa" if alt[0] % 2 else "sc"
            return psum.tile(shape, f32, tag=tag, bufs=2, name=name)

        for lt in range(LT):
            for which, xdram in (("q", xq), ("k", xk), ("v", xv)):
                x_sb = xs.tile([128, CC * 512], bf16, tag="xs",
                               name=f"x_{which}{lt}")
                for c2 in range(CC // 2):
                    nc.sync.dma_start(
                        x_sb[:, c2 * 1024:(c2 + 1) * 1024].rearrange(
                            "p (r w) -> p r w", r=2),
                        xdram[c2 * 256:(c2 + 1) * 256,
                              lt * 512:(lt + 1) * 512].rearrange(
                            "(r p) w -> p r w", p=128))
                if which in ("q", "k"):
                    w_sb = wq_sb if which == "q" else wk_sb
                    t_sb = qt_sb if which == "q" else kt_sb
                    for hp in range(HP):
                        ps = proj_ps([128, 512], f"psp_{which}{lt}_{hp}")
                        for c in range(CC):
                            nc.tensor.matmul(
                                ps[:],
                                lhsT=w_sb[:, c * DL + hp * 128: c * DL + hp * 128 + 128],
                                rhs=x_sb[:, c * 512:(c + 1) * 512],
                                start=(c == 0), stop=(c == CC - 1))
                        dst = t_sb[hp][:, lt * 512:(lt + 1) * 512]
                        if which == "q":
                            nc.scalar.activation(dst, ps[:], AF.Identity,
                                                 bias=bq_sb[:, hp:hp + 1], scale=1.0)
                        else:
                            nc.vector.tensor_copy(dst, ps[:])
                else:
                    for sub in range(4):
                        kcg = lt * 4 + sub
                        ps = proj_ps([128, DL], f"psp_v{kcg}")
                        for c in range(CC):
                            lhsT = x_sb[:, c * 512 + sub * 128: c * 512 + sub * 128 + 128]
                            nc.tensor.matmul(
                                ps[:], lhsT=lhsT,
                                rhs=wv_sb[:, c * DL:(c + 1) * DL],
                                start=(c == 0), stop=(c == CC - 1))
                        vbk = vb_sb[:, kcg * VW:(kcg + 1) * VW].rearrange(
                            "p (h w) -> p h w", w=65)
                        nc.vector.tensor_copy(
                            vbk[:, :, 0:64],
                            ps.rearrange("p (h w) -> p h w", w=64))

        # ================= Phase B: banded attention, q-tile passes =========
        # Flat software pipeline over all (qt, head-pair, kc-piece) units with
        # one-piece lookahead crossing pair and qt boundaries.  PSUM: "sc"
        # ring 2x2 banks (scores + outproj pf), "pa" ring 2x2 banks (Phase A
        # proj + po accumulators) = 8 banks.  The distance bias -0.8|k-q| is
        # added into the score PSUM by an identity matmul, so exp(0.125*x)
        # yields the biased weights directly (no separate DVE multiply).

        def outproj_qc(qc):
            st = ostage.tile([128, C], bf16, tag="fo", name=f"fo{qc}")
            pf = psum.tile([128, 1024], f32, tag="sc", bufs=2, name=f"pf{qc}")
            for mi, mo in enumerate((0, 512)):
                for hp2 in range(HP):
                    nc.tensor.matmul(
                        pf[:, mo:mo + 512],
                        lhsT=ots_sb[hp2][:, qc * 128:(qc + 1) * 128],
                        rhs=wo_sb[:, hp2 * C + mo: hp2 * C + mo + 512],
                        start=(hp2 == 0), stop=(hp2 == HP - 1))
            if qc % 2 == 0:
                nc.scalar.copy(st[:], pf[:])
            else:
                nc.vector.tensor_copy(st[:], pf[:])
            nc.gpsimd.dma_start(out[qc * 128:(qc + 1) * 128, :], st[:])

        units = []
        qt_start = {}
        for qt in range(NQT):
            pieces = cfg.pieces_of(qt)
            # greedy-pack consecutive kc-pieces into groups of total width
            # <=512 so each group needs one PSUM bank per head and one exp
            groups = []
            for kc, q0, n in pieces:
                if groups and groups[-1][-1][3] + groups[-1][-1][2] + n <= 512:
                    off = groups[-1][-1][3] + groups[-1][-1][2]
                    groups[-1].append((kc, q0, n, off))
                else:
                    groups.append([(kc, q0, n, 0)])
            qt_start[qt] = len(units)
            for hp in range(HP):
                for idx, subs in enumerate(groups):
                    units.append(dict(
                        qt=qt, hp=hp, subs=subs,
                        first=(idx == 0), last=(idx == len(groups) - 1)))
        # outproj(qt) is delayed into qt+1's unit stream (one qc per ~2
        # units) so the normalization chain overlaps flowing attention work
        # and the outproj matmuls act as PE filler
        after_unit = {}
        for qt in range(NQT):
            for j in range(4):
                qc = 4 * qt + j
                if qt + 1 < NQT:
                    key = qt_start[qt + 1] + 4 * j + 1
                else:
                    key = len(units) - 1
                after_unit.setdefault(key, []).append(qc)

        state = {}

        def scores(u):
            v = units[u]
            sc = psum.tile([128, 1024], f32, tag="sc", bufs=2, name=f"sc{u}")
            state[u] = sc
            for si, (kc, q0, n, off) in enumerate(v["subs"]):
                for hi in range(2):
                    nc.tensor.matmul(
                        sc[:, hi * 512 + off: hi * 512 + off + n],
                        lhsT=kt_sb[v["hp"]][hi * 64:(hi + 1) * 64,
                                            kc * 128:(kc + 1) * 128],
                        rhs=qt_sb[v["hp"]][hi * 64:(hi + 1) * 64, q0: q0 + n],
                        start=(si == 0), stop=False)
            last = len(v["subs"]) - 1
            for si, (kc, q0, n, off) in enumerate(v["subs"]):
                seb = q0 - 128 * kc + 512
                for hi in range(2):
                    nc.tensor.matmul(
                        sc[:, hi * 512 + off: hi * 512 + off + n],
                        lhsT=idn_sb[:],
                        rhs=eb_sb[:, seb:seb + n],
                        start=False, stop=(si == last))

        def finish(u):
            v = units[u]
            qt, hp = v["qt"], v["hp"]
            sc = state.pop(u)
            if v["first"]:
                state[("po", qt, hp)] = psum.tile(
                    [65, 1024], f32, tag="pa", bufs=2, name=f"po{qt}_{hp}")
            po = state[("po", qt, hp)]
            wg = v["subs"][-1][3] + v["subs"][-1][2]  # group width
            et = ets.tile([128, 1024], bf16, tag="et", name=f"et{u}")
            et2 = et.rearrange("p (r w) -> p r w", r=2)
            sc3 = sc.rearrange("p (r w) -> p r w", r=2)
            nc.scalar.activation(et2[:, :, 0:wg], sc3[:, :, 0:wg],
                                 AF.Exp, scale=0.125)
            last = len(v["subs"]) - 1
            for si, (kc, q0, n, off) in enumerate(v["subs"]):
                qoff = q0 - 512 * qt
                for hi in range(2):
                    h = 2 * hp + hi
                    vsl = vb_sb[:, kc * VW + h * 65: kc * VW + h * 65 + 65]
                    nc.tensor.matmul(
                        po[0:65, hi * 512 + qoff: hi * 512 + qoff + n],
                        lhsT=vsl,
                        rhs=et2[:, hi, off:off + n],
                        start=(v["first"] and si == 0),
                        stop=(v["last"] and si == last))
            if not v["last"]:
                return
            # normalize: denominators sit in po row 64 (both heads); stage to
            # partition 0 first (the custom-DVE recip can't cross 32-strips)
            po = state.pop(("po", qt, hp))
            s_st = rbp.tile([1, 1024], f32, tag="ss", name=f"ss{qt}_{hp}")
            if hp % 2 == 0:
                nc.scalar.copy(s_st[:], po[64:65, :])
            else:
                nc.vector.tensor_copy(s_st[:], po[64:65, :])
            r_f = rbp.tile([1, 1024], f32, tag="rf", name=f"rf{qt}_{hp}")
            nc.vector.reciprocal_approx_fast(r_f[:], s_st[:])
            rbb = rbp.tile([64, 1024], f32, tag="rbb", name=f"rbb{qt}_{hp}")
            nc.gpsimd.partition_broadcast(rbb[:], r_f[:])
            for hi in range(2):
                sl = (slice(hi * 64, (hi + 1) * 64),
                      slice(qt * 512, (qt + 1) * 512))
                nc.vector.tensor_mul(
                    ots_sb[hp][sl],
                    po[0:64, hi * 512: hi * 512 + 512],
                    rbb[:, hi * 512: hi * 512 + 512])

        scores(0)
        for u in range(1, len(units)):
            scores(u)
            finish(u - 1)
            for qc in after_unit.get(u - 1, ()):
                outproj_qc(qc)
        finish(len(units) - 1)
        for qc in after_unit.get(len(units) - 1, ()):
            outproj_qc(qc)

    nc.compile()
    return nc


def host_inputs(inputs, cfg=FULL):
    """Build the 8 per-core input maps + the host-side combine constant."""
    L, C, DL, NHL = cfg.L, cfg.C, cfg.DL, cfg.NHL
    q = np.asarray(inputs["queries"], np.float32)
    k = np.asarray(inputs["keys"], np.float32)
    v = np.asarray(inputs["values"], np.float32)
    Wq = np.asarray(inputs["Wq"], np.float32)
    Wk = np.asarray(inputs["Wk"], np.float32)
    Wv = np.asarray(inputs["Wv"], np.float32)
    Wo = np.asarray(inputs["Wo"], np.float32)
    bq = np.asarray(inputs["bq"], np.float32)
    bv = np.asarray(inputs["bv"], np.float32)
    bo = np.asarray(inputs["bo"], np.float32)
    B = q.shape[0]

    bo_eff = (bo.astype(np.float64) + Wo.astype(np.float64) @ bv.astype(np.float64)
              ).astype(np.float32)

    p = np.arange(128, dtype=np.float64)[:, None]
    c = np.arange(cfg.EBW, dtype=np.float64)[None, :]
    # additive log-bias, pre-divided by the 0.125 softmax scale:
    # exp(0.125*(s + eb)) = exp(s/8 - 0.1|k-q|)
    eb = (-0.8 * np.abs(p - c + 512)).astype(BF16)
    idn = np.eye(128, dtype=BF16)

    xT = {}
    for b in range(B):
        xT[b] = (np.ascontiguousarray(q[b].T).astype(BF16),
                 np.ascontiguousarray(k[b].T).astype(BF16),
                 np.ascontiguousarray(v[b].T).astype(BF16))

    in_maps = []
    for core in range(N_CORES):
        b, hg = core // 2, core % 2
        sl = slice(hg * DL, (hg + 1) * DL)
        in_maps.append({
            "xq": xT[b][0], "xk": xT[b][1], "xv": xT[b][2],
            "wq": np.ascontiguousarray(Wq.T[:, sl]).astype(BF16),
            "wk": np.ascontiguousarray(Wk.T[:, sl]).astype(BF16),
            "wv": np.ascontiguousarray(Wv.T[:, sl]).astype(BF16),
            "wo": np.ascontiguousarray(Wo.T[sl, :]).astype(BF16),
            "bq": np.ascontiguousarray(bq[sl][:, None]),
            "eb": eb, "idn": idn,
        })
    return in_maps, bo_eff


_CACHED = {}


def _wait_devices_healthy(timeout_s=420):
    import time
    import jax
    import jax.numpy as jnp
    t0 = time.time()
    last = None
    while time.time() - t0 < timeout_s:
        try:
            for d in jax.devices():
                x = jax.device_put(np.ones((8, 8), np.float32), d)
                jnp.sum(x).block_until_ready()
            return
        except Exception as e:  # wedged worker recycles within a few minutes
            last = e
            time.sleep(15)
    raise RuntimeError(f"NeuronCores unhealthy after {timeout_s}s: {last}")


def kernel(**inputs):
    from concourse.bass_utils import run_bass_kernel_spmd

    cfg = FULL
    if "nc" not in _CACHED:
        _CACHED["nc"] = build_program(cfg)
    nc = _CACHED["nc"]

    in_maps, bo_eff = host_inputs(inputs, cfg)
    _wait_devices_healthy()
    try:
        res = run_bass_kernel_spmd(nc, in_maps, core_ids=list(range(N_CORES)))
    except Exception:
        _wait_devices_healthy()
        res = run_bass_kernel_spmd(nc, in_maps, core_ids=list(range(N_CORES)))
    B = np.asarray(inputs["queries"]).shape[0]
    out = np.zeros((B, cfg.L, cfg.C), np.float32)
    for b in range(B):
        out[b] = (res.results[2 * b]["out"].astype(np.float32)
                  + res.results[2 * b + 1]["out"].astype(np.float32)
                  + bo_eff[None, :])
    return out


# revision 38
# speedup vs baseline: 1.2051x; 1.1718x over previous
"""Trainium2 Bass kernel for nn_AutoCorrelation (multi-head attention with a
distance decay bias), SPMD across 8 NeuronCores.

Sharding: core = (batch b, head-group hg) with b in 0..3, hg in 0..1.
Each core computes, for its batch and its 8 heads: QKV projections
(column-sharded weights), distance-banded attention (the -0.1*|i-j| bias makes
weights beyond |i-j|~96 numerically zero), and a row-sharded output
projection. The host sums the two half partial outputs per batch and adds the
effective output bias.

Math notes:
 - bk drops out entirely (softmax row-shift invariance); bv is folded into the
   host-side output bias: bo_eff = bo + Wo @ bv.
 - scores are built transposed St[k, q]; a ones-column appended to V yields the
   softmax denominators in the same matmul (PSUM row 64).
 - the bias exp(-0.1|k-q|) is a Toeplitz multiply from a precomputed master.

Structure (v2):
 - warmup matmuls at t=0 keep the PE HAM un-throttled while input DMA streams;
   weight DMAs issue from gpsimd in parallel with x loads on sync.
 - Phase B runs as 4 q-tile passes; per pass all 8 heads are processed as 4
   pairs with concurrent K=64 score matmuls (tile_position row groups), one
   merged exp ACTIVATE per pair, and the output projection + out DMA run
   inside the pass so HBM writes spread across the kernel.
"""

import math
from contextlib import ExitStack

import numpy as np
import ml_dtypes

BF16 = ml_dtypes.bfloat16

N_CORES = 8


class Cfg:
    def __init__(self, L=2048, C=1024, NHL=8, DK=64, W=96):
        self.L, self.C, self.NHL, self.DK, self.W = L, C, NHL, DK, W
        self.DL = NHL * DK               # local head dims
        self.SPAN = 128 + 2 * W          # k-chunk q-span
        self.KC = L // 128               # k chunks
        self.NQT = L // 512              # q tiles (512)
        self.CC = C // 128               # contraction chunks
        self.LT = L // 512               # l tiles
        self.HP = NHL // 2               # head pairs
        self.VW = NHL * 65               # padded V width
        self.EBW = self.SPAN + 576       # EB master width (seb range + span)
        assert self.SPAN <= self.L

    def qs_of(self, kc):
        return min(max(128 * kc - self.W, 0), self.L - self.SPAN)

    def pieces_of(self, qt):
        """(kc, q0, N) pieces whose span intersects q-tile qt."""
        lo, hi = 512 * qt, 512 * qt + 512
        out = []
        for kc in range(self.KC):
            qs = self.qs_of(kc)
            q0, q1 = max(qs, lo), min(qs + self.SPAN, hi)
            if q1 > q0:
                out.append((kc, q0, q1 - q0))
        return out


FULL = Cfg(W=80)


def build_program(cfg=FULL, debug=False):
    import concourse.bass as bass
    import concourse.tile as tile
    from concourse import bacc, mybir

    f32 = mybir.dt.float32
    bf16 = mybir.dt.bfloat16
    AF = mybir.ActivationFunctionType

    L, C, NHL, DL = cfg.L, cfg.C, cfg.NHL, cfg.DL
    KC, NQT, CC, LT, HP, VW = cfg.KC, cfg.NQT, cfg.CC, cfg.LT, cfg.HP, cfg.VW

    nc = bacc.Bacc("TRN2", target_bir_lowering=False, debug=debug,
                   num_devices=N_CORES)

    xq = nc.dram_tensor("xq", [C, L], bf16, kind="ExternalInput").ap()
    xk = nc.dram_tensor("xk", [C, L], bf16, kind="ExternalInput").ap()
    xv = nc.dram_tensor("xv", [C, L], bf16, kind="ExternalInput").ap()
    wq = nc.dram_tensor("wq", [C, DL], bf16, kind="ExternalInput").ap()
    wk = nc.dram_tensor("wk", [C, DL], bf16, kind="ExternalInput").ap()
    wv = nc.dram_tensor("wv", [C, DL], bf16, kind="ExternalInput").ap()
    wo = nc.dram_tensor("wo", [DL, C], bf16, kind="ExternalInput").ap()
    bqd = nc.dram_tensor("bq", [DL, 1], f32, kind="ExternalInput").ap()
    ebd = nc.dram_tensor("eb", [128, cfg.EBW], bf16, kind="ExternalInput").ap()
    idnd = nc.dram_tensor("idn", [128, 128], bf16, kind="ExternalInput").ap()
    out = nc.dram_tensor("out", [L, C], bf16, kind="ExternalOutput").ap()

    with tile.TileContext(nc) as tc, ExitStack() as ctx:
        const = ctx.enter_context(tc.tile_pool(name="const", bufs=1))
        big = ctx.enter_context(tc.tile_pool(name="big", bufs=1))
        xs = ctx.enter_context(tc.tile_pool(name="xs", bufs=3))
        ets = ctx.enter_context(tc.tile_pool(name="ets", bufs=4))
        rbp = ctx.enter_context(tc.tile_pool(name="rbp", bufs=2))
        ostage = ctx.enter_context(tc.tile_pool(name="ostage", bufs=2))
        psum = ctx.enter_context(tc.tile_pool(name="psum", bufs=1, space="PSUM"))

        # ---- PE warmup: keep HAM un-throttled while input DMA streams ----
        dummy = const.tile([128, 512], bf16)
        nc.vector.memset(dummy[:], 0.0)
        # dummy exp pulls the ACT table load off the critical path
        dummy_act = const.tile([1, 8], f32)
        nc.scalar.activation(dummy_act[:], dummy[0:1, 0:8], AF.Exp, scale=0.125)
        warm_ps = psum.tile([128, 1024], f32, tag="pa", bufs=2, name="warm")
        for i in range(28):
            nc.tensor.matmul(warm_ps[:, 0:512], lhsT=dummy[:, 0:128],
                             rhs=dummy[:], start=True, stop=True)
        warm_sink = const.tile([128, 512], bf16)
        nc.vector.tensor_copy(warm_sink[:], warm_ps[:, 0:512])

        # ---- resident constants (weight DMAs on gpsimd, x loads on sync) ----
        wq_sb = const.tile([128, CC * DL], bf16)
        wk_sb = const.tile([128, CC * DL], bf16)
        wv_sb = const.tile([128, CC * DL], bf16)
        wo_sb = const.tile([128, HP * C], bf16)
        bq_sb = const.tile([128, HP], f32)
        for hp in range(HP):
            nc.gpsimd.dma_start(bq_sb[:, hp:hp + 1], bqd[hp * 128:(hp + 1) * 128, :])
        # two 128-row chunks per descriptor: halves the ~600ns-per-dma_start
        # issue cost that was starving the early projection matmuls
        def load_w2(dst_sb, src):
            for c2 in range(CC // 2):
                nc.gpsimd.dma_start(
                    dst_sb[:, c2 * 2 * DL:(c2 + 1) * 2 * DL].rearrange(
                        "p (r w) -> p r w", r=2),
                    src[c2 * 256:(c2 + 1) * 256, :].rearrange(
                        "(r p) w -> p r w", p=128))

        load_w2(wq_sb, wq)
        load_w2(wk_sb, wk)
        load_w2(wv_sb, wv)
        eb_sb = const.tile([128, cfg.EBW], bf16)
        nc.gpsimd.dma_start(eb_sb[:], ebd[:])
        idn_sb = const.tile([128, 128], bf16)
        nc.gpsimd.dma_start(idn_sb[:], idnd[:])
        for hp in range(HP):
            nc.gpsimd.dma_start(wo_sb[:, hp * C:(hp + 1) * C], wo[hp * 128:(hp + 1) * 128, :])

        # ---- resident activations ----
        qt_sb = [big.tile([128, L], bf16, name=f"qt{hp}") for hp in range(HP)]
        kt_sb = [big.tile([128, L], bf16, name=f"kt{hp}") for hp in range(HP)]
        vb_sb = big.tile([128, KC * VW], bf16)
        ots_sb = [big.tile([128, L], bf16, name=f"ots{hp}") for hp in range(HP)]

        # ones columns of the [V_h | ones] layout, written once on DVE
        for kcg in range(KC):
            vbk = vb_sb[:, kcg * VW:(kcg + 1) * VW].rearrange(
                "p (h w) -> p h w", w=65)
            nc.vector.memset(vbk[:, :, 64:65], 1.0)

        # ================= Phase A: projections =================
        # Alternate projection PSUM groups across both ring tags ("pa"/"sc")
        # for an effective 4-deep pipeline: the drain of group i no longer
        # gates the matmuls of group i+2.
        alt = [0]

        def proj_ps(shape, name):
            alt[0] += 1
            tag = "pa" if alt[0] % 2 else "sc"
            return psum.tile(shape, f32, tag=tag, bufs=2, name=name)

        for lt in range(LT):
            for which, xdram in (("q", xq), ("k", xk), ("v", xv)):
                x_sb = xs.tile([128, CC * 512], bf16, tag="xs",
                               name=f"x_{which}{lt}")
                for c2 in range(CC // 2):
                    nc.sync.dma_start(
                        x_sb[:, c2 * 1024:(c2 + 1) * 1024].rearrange(
                            "p (r w) -> p r w", r=2),
                        xdram[c2 * 256:(c2 + 1) * 256,
                              lt * 512:(lt + 1) * 512].rearrange(
                            "(r p) w -> p r w", p=128))
                if which in ("q", "k"):
                    w_sb = wq_sb if which == "q" else wk_sb
                    t_sb = qt_sb if which == "q" else kt_sb
                    for hp in range(HP):
                        ps = proj_ps([128, 512], f"psp_{which}{lt}_{hp}")
                        for c in range(CC):
                            nc.tensor.matmul(
                                ps[:],
                                lhsT=w_sb[:, c * DL + hp * 128: c * DL + hp * 128 + 128],
                                rhs=x_sb[:, c * 512:(c + 1) * 512],
                                start=(c == 0), stop=(c == CC - 1))
                        dst = t_sb[hp][:, lt * 512:(lt + 1) * 512]
                        if which == "q":
                            nc.scalar.activation(dst, ps[:], AF.Identity,
                                                 bias=bq_sb[:, hp:hp + 1], scale=1.0)
                        else:
                            nc.vector.tensor_copy(dst, ps[:])
                else:
                    for sub in range(4):
                        kcg = lt * 4 + sub
                        ps = proj_ps([128, DL], f"psp_v{kcg}")
                        for c in range(CC):
                            lhsT = x_sb[:, c * 512 + sub * 128: c * 512 + sub * 128 + 128]
                            nc.tensor.matmul(
                                ps[:], lhsT=lhsT,
                                rhs=wv_sb[:, c * DL:(c + 1) * DL],
                                start=(c == 0), stop=(c == CC - 1))
                        vbk = vb_sb[:, kcg * VW:(kcg + 1) * VW].rearrange(
                            "p (h w) -> p h w", w=65)
                        nc.vector.tensor_copy(
                            vbk[:, :, 0:64],
                            ps.rearrange("p (h w) -> p h w", w=64))

        # ================= Phase B: banded attention, q-tile passes =========
        # Flat software pipeline over all (qt, head-pair, kc-piece) units with
        # one-piece lookahead crossing pair and qt boundaries.  PSUM: "sc"
        # ring 2x2 banks (scores + outproj pf), "pa" ring 2x2 banks (Phase A
        # proj + po accumulators) = 8 banks.  The distance bias -0.8|k-q| is
        # added into the score PSUM by an identity matmul, so exp(0.125*x)
        # yields the biased weights directly (no separate DVE multiply).

        def outproj_qc(qc):
            st = ostage.tile([128, C], bf16, tag="fo", name=f"fo{qc}")
            pf = psum.tile([128, 1024], f32, tag="sc", bufs=2, name=f"pf{qc}")
            for mi, mo in enumerate((0, 512)):
                for hp2 in range(HP):
                    nc.tensor.matmul(
                        pf[:, mo:mo + 512],
                        lhsT=ots_sb[hp2][:, qc * 128:(qc + 1) * 128],
                        rhs=wo_sb[:, hp2 * C + mo: hp2 * C + mo + 512],
                        start=(hp2 == 0), stop=(hp2 == HP - 1))
            if qc % 2 == 0:
                nc.scalar.copy(st[:], pf[:])
            else:
                nc.vector.tensor_copy(st[:], pf[:])
            nc.gpsimd.dma_start(out[qc * 128:(qc + 1) * 128, :], st[:])

        units = []
        qt_start = {}
        for qt in range(NQT):
            pieces = cfg.pieces_of(qt)
            # greedy-pack consecutive kc-pieces into groups of total width
            # <=512 so each group needs one PSUM bank per head and one exp
            groups = []
            for kc, q0, n in pieces:
                if groups and groups[-1][-1][3] + groups[-1][-1][2] + n <= 512:
                    off = groups[-1][-1][3] + groups[-1][-1][2]
                    groups[-1].append((kc, q0, n, off))
                else:
                    groups.append([(kc, q0, n, 0)])
            qt_start[qt] = len(units)
            for hp in range(HP):
                for idx, subs in enumerate(groups):
                    units.append(dict(
                        qt=qt, hp=hp, subs=subs,
                        first=(idx == 0), last=(idx == len(groups) - 1)))
        # outproj(qt) is delayed into qt+1's unit stream (one qc per ~2
        # units) so the normalization chain overlaps flowing attention work
        # and the outproj matmuls act as PE filler
        after_unit = {}
        for qt in range(NQT):
            for j in range(4):
                qc = 4 * qt + j
                if qt + 1 < NQT:
                    key = qt_start[qt + 1] + 4 * j + 1
                else:
                    key = len(units) - 1
                after_unit.setdefault(key, []).append(qc)

        state = {}

        def scores(u):
            v = units[u]
            sc = psum.tile([128, 1024], f32, tag="sc", bufs=2, name=f"sc{u}")
            state[u] = sc
            for si, (kc, q0, n, off) in enumerate(v["subs"]):
                for hi in range(2):
                    nc.tensor.matmul(
                        sc[:, hi * 512 + off: hi * 512 + off + n],
                        lhsT=kt_sb[v["hp"]][hi * 64:(hi + 1) * 64,
                                            kc * 128:(kc + 1) * 128],
                        rhs=qt_sb[v["hp"]][hi * 64:(hi + 1) * 64, q0: q0 + n],
                        start=(si == 0), stop=False)
            last = len(v["subs"]) - 1
            for si, (kc, q0, n, off) in enumerate(v["subs"]):
                seb = q0 - 128 * kc + 512
                for hi in range(2):
                    nc.tensor.matmul(
                        sc[:, hi * 512 + off: hi * 512 + off + n],
                        lhsT=idn_sb[:],
                        rhs=eb_sb[:, seb:seb + n],
                        start=False, stop=(si == last))

        def expp(u):
            # exp runs one pipeline step behind scores; PV two steps behind,
            # so the PE fills the ACT latency with PV work instead of
            # stalling right after the next unit's score matmuls
            v = units[u]
            sc = state.pop(u)
            wg = v["subs"][-1][3] + v["subs"][-1][2]  # group width
            et = ets.tile([128, 1024], bf16, tag="et", name=f"et{u}")
            et2 = et.rearrange("p (r w) -> p r w", r=2)
            sc3 = sc.rearrange("p (r w) -> p r w", r=2)
            nc.scalar.activation(et2[:, :, 0:wg], sc3[:, :, 0:wg],
                                 AF.Exp, scale=0.125)
            state[("et", u)] = et2

        def pvpart(u):
            v = units[u]
            qt, hp = v["qt"], v["hp"]
            et2 = state.pop(("et", u))
            if v["first"]:
                state[("po", qt, hp)] = psum.tile(
                    [65, 1024], f32, tag="pa", bufs=2, name=f"po{qt}_{hp}")
            po = state[("po", qt, hp)]
            last = len(v["subs"]) - 1
            for si, (kc, q0, n, off) in enumerate(v["subs"]):
                qoff = q0 - 512 * qt
                for hi in range(2):
                    h = 2 * hp + hi
                    vsl = vb_sb[:, kc * VW + h * 65: kc * VW + h * 65 + 65]
                    nc.tensor.matmul(
                        po[0:65, hi * 512 + qoff: hi * 512 + qoff + n],
                        lhsT=vsl,
                        rhs=et2[:, hi, off:off + n],
                        start=(v["first"] and si == 0),
                        stop=(v["last"] and si == last))
            if not v["last"]:
                return
            # normalize: denominators sit in po row 64 (both heads); stage to
            # partition 0 first (the custom-DVE recip can't cross 32-strips)
            po = state.pop(("po", qt, hp))
            s_st = rbp.tile([1, 1024], f32, tag="ss", name=f"ss{qt}_{hp}")
            if hp % 2 == 0:
                nc.scalar.copy(s_st[:], po[64:65, :])
            else:
                nc.vector.tensor_copy(s_st[:], po[64:65, :])
            r_f = rbp.tile([1, 1024], f32, tag="rf", name=f"rf{qt}_{hp}")
            nc.vector.reciprocal_approx_fast(r_f[:], s_st[:])
            rbb = rbp.tile([64, 1024], f32, tag="rbb", name=f"rbb{qt}_{hp}")
            nc.gpsimd.partition_broadcast(rbb[:], r_f[:])
            for hi in range(2):
                sl = (slice(hi * 64, (hi + 1) * 64),
                      slice(qt * 512, (qt + 1) * 512))
                nc.vector.tensor_mul(
                    ots_sb[hp][sl],
                    po[0:64, hi * 512: hi * 512 + 512],
                    rbb[:, hi * 512: hi * 512 + 512])

        U = len(units)
        scores(0)
        scores(1)
        expp(0)
        for u in range(2, U):
            scores(u)
            expp(u - 1)
            pvpart(u - 2)
            for qc in after_unit.get(u - 2, ()):
                outproj_qc(qc)
        expp(U - 1)
        for u in (U - 2, U - 1):
            pvpart(u)
            for qc in after_unit.get(u, ()):
                outproj_qc(qc)

    nc.compile()
    return nc


def host_inputs(inputs, cfg=FULL):
    """Build the 8 per-core input maps + the host-side combine constant."""
    L, C, DL, NHL = cfg.L, cfg.C, cfg.DL, cfg.NHL
    q = np.asarray(inputs["queries"], np.float32)
    k = np.asarray(inputs["keys"], np.float32)
    v = np.asarray(inputs["values"], np.float32)
    Wq = np.asarray(inputs["Wq"], np.float32)
    Wk = np.asarray(inputs["Wk"], np.float32)
    Wv = np.asarray(inputs["Wv"], np.float32)
    Wo = np.asarray(inputs["Wo"], np.float32)
    bq = np.asarray(inputs["bq"], np.float32)
    bv = np.asarray(inputs["bv"], np.float32)
    bo = np.asarray(inputs["bo"], np.float32)
    B = q.shape[0]

    bo_eff = (bo.astype(np.float64) + Wo.astype(np.float64) @ bv.astype(np.float64)
              ).astype(np.float32)

    p = np.arange(128, dtype=np.float64)[:, None]
    c = np.arange(cfg.EBW, dtype=np.float64)[None, :]
    # additive log-bias, pre-divided by the 0.125 softmax scale:
    # exp(0.125*(s + eb)) = exp(s/8 - 0.1|k-q|)
    eb = (-0.8 * np.abs(p - c + 512)).astype(BF16)
    idn = np.eye(128, dtype=BF16)

    xT = {}
    for b in range(B):
        xT[b] = (np.ascontiguousarray(q[b].T).astype(BF16),
                 np.ascontiguousarray(k[b].T).astype(BF16),
                 np.ascontiguousarray(v[b].T).astype(BF16))

    in_maps = []
    for core in range(N_CORES):
        b, hg = core // 2, core % 2
        sl = slice(hg * DL, (hg + 1) * DL)
        in_maps.append({
            "xq": xT[b][0], "xk": xT[b][1], "xv": xT[b][2],
            "wq": np.ascontiguousarray(Wq.T[:, sl]).astype(BF16),
            "wk": np.ascontiguousarray(Wk.T[:, sl]).astype(BF16),
            "wv": np.ascontiguousarray(Wv.T[:, sl]).astype(BF16),
            "wo": np.ascontiguousarray(Wo.T[sl, :]).astype(BF16),
            "bq": np.ascontiguousarray(bq[sl][:, None]),
            "eb": eb, "idn": idn,
        })
    return in_maps, bo_eff


_CACHED = {}


def _wait_devices_healthy(timeout_s=420):
    import time
    import jax
    import jax.numpy as jnp
    t0 = time.time()
    last = None
    while time.time() - t0 < timeout_s:
        try:
            for d in jax.devices():
                x = jax.device_put(np.ones((8, 8), np.float32), d)
                jnp.sum(x).block_until_ready()
            return
        except Exception as e:  # wedged worker recycles within a few minutes
            last = e
            time.sleep(15)
    raise RuntimeError(f"NeuronCores unhealthy after {timeout_s}s: {last}")


def kernel(**inputs):
    from concourse.bass_utils import run_bass_kernel_spmd

    cfg = FULL
    if "nc" not in _CACHED:
        _CACHED["nc"] = build_program(cfg)
    nc = _CACHED["nc"]

    in_maps, bo_eff = host_inputs(inputs, cfg)
    _wait_devices_healthy()
    try:
        res = run_bass_kernel_spmd(nc, in_maps, core_ids=list(range(N_CORES)))
    except Exception:
        _wait_devices_healthy()
        res = run_bass_kernel_spmd(nc, in_maps, core_ids=list(range(N_CORES)))
    B = np.asarray(inputs["queries"]).shape[0]
    out = np.zeros((B, cfg.L, cfg.C), np.float32)
    for b in range(B):
        out[b] = (res.results[2 * b]["out"].astype(np.float32)
                  + res.results[2 * b + 1]["out"].astype(np.float32)
                  + bo_eff[None, :])
    return out


# revision 40
# speedup vs baseline: 1.2095x; 1.0036x over previous
"""Trainium2 Bass kernel for nn_AutoCorrelation (multi-head attention with a
distance decay bias), SPMD across 8 NeuronCores.

Sharding: core = (batch b, head-group hg) with b in 0..3, hg in 0..1.
Each core computes, for its batch and its 8 heads: QKV projections
(column-sharded weights), distance-banded attention (the -0.1*|i-j| bias makes
weights beyond |i-j|~96 numerically zero), and a row-sharded output
projection. The host sums the two half partial outputs per batch and adds the
effective output bias.

Math notes:
 - bk drops out entirely (softmax row-shift invariance); bv is folded into the
   host-side output bias: bo_eff = bo + Wo @ bv.
 - scores are built transposed St[k, q]; a ones-column appended to V yields the
   softmax denominators in the same matmul (PSUM row 64).
 - the bias exp(-0.1|k-q|) is a Toeplitz multiply from a precomputed master.

Structure (v2):
 - warmup matmuls at t=0 keep the PE HAM un-throttled while input DMA streams;
   weight DMAs issue from gpsimd in parallel with x loads on sync.
 - Phase B runs as 4 q-tile passes; per pass all 8 heads are processed as 4
   pairs with concurrent K=64 score matmuls (tile_position row groups), one
   merged exp ACTIVATE per pair, and the output projection + out DMA run
   inside the pass so HBM writes spread across the kernel.
"""

import math
from contextlib import ExitStack

import numpy as np
import ml_dtypes

BF16 = ml_dtypes.bfloat16

N_CORES = 8


class Cfg:
    def __init__(self, L=2048, C=1024, NHL=8, DK=64, W=96):
        self.L, self.C, self.NHL, self.DK, self.W = L, C, NHL, DK, W
        self.DL = NHL * DK               # local head dims
        self.SPAN = 128 + 2 * W          # k-chunk q-span
        self.KC = L // 128               # k chunks
        self.NQT = L // 512              # q tiles (512)
        self.CC = C // 128               # contraction chunks
        self.LT = L // 512               # l tiles
        self.HP = NHL // 2               # head pairs
        self.VW = NHL * 65               # padded V width
        self.EBW = self.SPAN + 576       # EB master width (seb range + span)
        assert self.SPAN <= self.L

    def qs_of(self, kc):
        return min(max(128 * kc - self.W, 0), self.L - self.SPAN)

    def pieces_of(self, qt):
        """(kc, q0, N) pieces whose span intersects q-tile qt."""
        lo, hi = 512 * qt, 512 * qt + 512
        out = []
        for kc in range(self.KC):
            qs = self.qs_of(kc)
            q0, q1 = max(qs, lo), min(qs + self.SPAN, hi)
            if q1 > q0:
                out.append((kc, q0, q1 - q0))
        return out


FULL = Cfg(W=80)


def build_program(cfg=FULL, debug=False):
    import concourse.bass as bass
    import concourse.tile as tile
    from concourse import bacc, mybir

    f32 = mybir.dt.float32
    bf16 = mybir.dt.bfloat16
    AF = mybir.ActivationFunctionType

    L, C, NHL, DL = cfg.L, cfg.C, cfg.NHL, cfg.DL
    KC, NQT, CC, LT, HP, VW = cfg.KC, cfg.NQT, cfg.CC, cfg.LT, cfg.HP, cfg.VW

    nc = bacc.Bacc("TRN2", target_bir_lowering=False, debug=debug,
                   num_devices=N_CORES)

    xq = nc.dram_tensor("xq", [C, L], bf16, kind="ExternalInput").ap()
    xk = nc.dram_tensor("xk", [C, L], bf16, kind="ExternalInput").ap()
    xv = nc.dram_tensor("xv", [C, L], bf16, kind="ExternalInput").ap()
    wq = nc.dram_tensor("wq", [C, DL], bf16, kind="ExternalInput").ap()
    wk = nc.dram_tensor("wk", [C, DL], bf16, kind="ExternalInput").ap()
    wv = nc.dram_tensor("wv", [C, DL], bf16, kind="ExternalInput").ap()
    wo = nc.dram_tensor("wo", [DL, C], bf16, kind="ExternalInput").ap()
    bqd = nc.dram_tensor("bq", [DL, 1], f32, kind="ExternalInput").ap()
    ebd = nc.dram_tensor("eb", [128, cfg.EBW], bf16, kind="ExternalInput").ap()
    idnd = nc.dram_tensor("idn", [128, 128], bf16, kind="ExternalInput").ap()
    out = nc.dram_tensor("out", [L, C], bf16, kind="ExternalOutput").ap()

    with tile.TileContext(nc) as tc, ExitStack() as ctx:
        const = ctx.enter_context(tc.tile_pool(name="const", bufs=1))
        big = ctx.enter_context(tc.tile_pool(name="big", bufs=1))
        xs = ctx.enter_context(tc.tile_pool(name="xs", bufs=3))
        ets = ctx.enter_context(tc.tile_pool(name="ets", bufs=4))
        rbp = ctx.enter_context(tc.tile_pool(name="rbp", bufs=2))
        ostage = ctx.enter_context(tc.tile_pool(name="ostage", bufs=2))
        psum = ctx.enter_context(tc.tile_pool(name="psum", bufs=1, space="PSUM"))

        # ---- PE warmup: keep HAM un-throttled while input DMA streams ----
        dummy = const.tile([128, 512], bf16)
        nc.vector.memset(dummy[:], 0.0)
        # dummy exp pulls the ACT table load off the critical path
        dummy_act = const.tile([1, 8], f32)
        nc.scalar.activation(dummy_act[:], dummy[0:1, 0:8], AF.Exp, scale=0.125)
        warm_ps = psum.tile([128, 1024], f32, tag="pa", bufs=2, name="warm")
        for i in range(28):
            nc.tensor.matmul(warm_ps[:, 0:512], lhsT=dummy[:, 0:128],
                             rhs=dummy[:], start=True, stop=True)
        warm_sink = const.tile([128, 512], bf16)
        nc.vector.tensor_copy(warm_sink[:], warm_ps[:, 0:512])

        # ---- resident constants (weight DMAs on gpsimd, x loads on sync) ----
        wq_sb = const.tile([128, CC * DL], bf16)
        wk_sb = const.tile([128, CC * DL], bf16)
        wv_sb = const.tile([128, CC * DL], bf16)
        wo_sb = const.tile([128, HP * C], bf16)
        bq_sb = const.tile([128, HP], f32)
        for hp in range(HP):
            nc.gpsimd.dma_start(bq_sb[:, hp:hp + 1], bqd[hp * 128:(hp + 1) * 128, :])
        # two 128-row chunks per descriptor: halves the ~600ns-per-dma_start
        # issue cost that was starving the early projection matmuls
        def load_w2(dst_sb, src):
            for c2 in range(CC // 2):
                nc.gpsimd.dma_start(
                    dst_sb[:, c2 * 2 * DL:(c2 + 1) * 2 * DL].rearrange(
                        "p (r w) -> p r w", r=2),
                    src[c2 * 256:(c2 + 1) * 256, :].rearrange(
                        "(r p) w -> p r w", p=128))

        load_w2(wq_sb, wq)
        load_w2(wk_sb, wk)
        load_w2(wv_sb, wv)
        eb_sb = const.tile([128, cfg.EBW], bf16)
        nc.gpsimd.dma_start(eb_sb[:], ebd[:])
        idn_sb = const.tile([128, 128], bf16)
        nc.gpsimd.dma_start(idn_sb[:], idnd[:])
        for hp in range(HP):
            nc.gpsimd.dma_start(wo_sb[:, hp * C:(hp + 1) * C], wo[hp * 128:(hp + 1) * 128, :])

        # ---- resident activations ----
        qt_sb = [big.tile([128, L], bf16, name=f"qt{hp}") for hp in range(HP)]
        kt_sb = [big.tile([128, L], bf16, name=f"kt{hp}") for hp in range(HP)]
        vb_sb = big.tile([128, KC * VW], bf16)
        ots_sb = [big.tile([128, L], bf16, name=f"ots{hp}") for hp in range(HP)]

        # ones columns of the [V_h | ones] layout, written once on DVE
        for kcg in range(KC):
            vbk = vb_sb[:, kcg * VW:(kcg + 1) * VW].rearrange(
                "p (h w) -> p h w", w=65)
            nc.vector.memset(vbk[:, :, 64:65], 1.0)

        # ================= Phase A: projections =================
        # Alternate projection PSUM groups across both ring tags ("pa"/"sc")
        # for an effective 4-deep pipeline: the drain of group i no longer
        # gates the matmuls of group i+2.
        alt = [0]

        def proj_ps(shape, name):
            alt[0] += 1
            tag = "pa" if alt[0] % 2 else "sc"
            return psum.tile(shape, f32, tag=tag, bufs=2, name=name)

        for lt in range(LT):
            for which, xdram in (("q", xq), ("k", xk), ("v", xv)):
                x_sb = xs.tile([128, CC * 512], bf16, tag="xs",
                               name=f"x_{which}{lt}")
                for c2 in range(CC // 2):
                    nc.sync.dma_start(
                        x_sb[:, c2 * 1024:(c2 + 1) * 1024].rearrange(
                            "p (r w) -> p r w", r=2),
                        xdram[c2 * 256:(c2 + 1) * 256,
                              lt * 512:(lt + 1) * 512].rearrange(
                            "(r p) w -> p r w", p=128))
                if which in ("q", "k"):
                    w_sb = wq_sb if which == "q" else wk_sb
                    t_sb = qt_sb if which == "q" else kt_sb
                    for hp in range(HP):
                        ps = proj_ps([128, 512], f"psp_{which}{lt}_{hp}")
                        for c in range(CC):
                            nc.tensor.matmul(
                                ps[:],
                                lhsT=w_sb[:, c * DL + hp * 128: c * DL + hp * 128 + 128],
                                rhs=x_sb[:, c * 512:(c + 1) * 512],
                                start=(c == 0), stop=(c == CC - 1))
                        dst = t_sb[hp][:, lt * 512:(lt + 1) * 512]
                        if which == "q":
                            nc.scalar.activation(dst, ps[:], AF.Identity,
                                                 bias=bq_sb[:, hp:hp + 1], scale=1.0)
                        else:
                            nc.vector.tensor_copy(dst, ps[:])
                else:
                    for sub in range(4):
                        kcg = lt * 4 + sub
                        ps = proj_ps([128, DL], f"psp_v{kcg}")
                        for c in range(CC):
                            lhsT = x_sb[:, c * 512 + sub * 128: c * 512 + sub * 128 + 128]
                            nc.tensor.matmul(
                                ps[:], lhsT=lhsT,
                                rhs=wv_sb[:, c * DL:(c + 1) * DL],
                                start=(c == 0), stop=(c == CC - 1))
                        vbk = vb_sb[:, kcg * VW:(kcg + 1) * VW].rearrange(
                            "p (h w) -> p h w", w=65)
                        nc.vector.tensor_copy(
                            vbk[:, :, 0:64],
                            ps.rearrange("p (h w) -> p h w", w=64))

        # ================= Phase B: banded attention, q-tile passes =========
        # Flat software pipeline over all (qt, head-pair, kc-piece) units with
        # one-piece lookahead crossing pair and qt boundaries.  PSUM: "sc"
        # ring 2x2 banks (scores + outproj pf), "pa" ring 2x2 banks (Phase A
        # proj + po accumulators) = 8 banks.  The distance bias -0.8|k-q| is
        # added into the score PSUM by an identity matmul, so exp(0.125*x)
        # yields the biased weights directly (no separate DVE multiply).

        def outproj_qc(qc):
            st = ostage.tile([128, C], bf16, tag="fo", name=f"fo{qc}")
            pf = psum.tile([128, 1024], f32, tag="sc", bufs=2, name=f"pf{qc}")
            for mi, mo in enumerate((0, 512)):
                for hp2 in range(HP):
                    nc.tensor.matmul(
                        pf[:, mo:mo + 512],
                        lhsT=ots_sb[hp2][:, qc * 128:(qc + 1) * 128],
                        rhs=wo_sb[:, hp2 * C + mo: hp2 * C + mo + 512],
                        start=(hp2 == 0), stop=(hp2 == HP - 1))
            if qc % 2 == 0:
                nc.scalar.copy(st[:], pf[:])
            else:
                nc.vector.tensor_copy(st[:], pf[:])
            nc.gpsimd.dma_start(out[qc * 128:(qc + 1) * 128, :], st[:])

        units = []
        qt_start = {}
        for qt in range(NQT):
            pieces = cfg.pieces_of(qt)
            # greedy-pack consecutive kc-pieces into groups of total width
            # <=512 so each group needs one PSUM bank per head and one exp
            groups = []
            for kc, q0, n in pieces:
                if groups and groups[-1][-1][3] + groups[-1][-1][2] + n <= 512:
                    off = groups[-1][-1][3] + groups[-1][-1][2]
                    groups[-1].append((kc, q0, n, off))
                else:
                    groups.append([(kc, q0, n, 0)])
            qt_start[qt] = len(units)
            for hp in range(HP):
                for idx, subs in enumerate(groups):
                    units.append(dict(
                        qt=qt, hp=hp, subs=subs,
                        first=(idx == 0), last=(idx == len(groups) - 1)))
        # outproj(qt) is delayed into qt+1's unit stream (one qc per ~2
        # units) so the normalization chain overlaps flowing attention work
        # and the outproj matmuls act as PE filler
        after_unit = {}
        for qt in range(NQT):
            for j in range(4):
                qc = 4 * qt + j
                if qt + 1 < NQT:
                    key = qt_start[qt + 1] + 4 * j + 2
                else:
                    key = len(units) - 1
                after_unit.setdefault(key, []).append(qc)

        state = {}

        def scores(u):
            v = units[u]
            sc = psum.tile([128, 1024], f32, tag="sc", bufs=2, name=f"sc{u}")
            state[u] = sc
            for si, (kc, q0, n, off) in enumerate(v["subs"]):
                for hi in range(2):
                    nc.tensor.matmul(
                        sc[:, hi * 512 + off: hi * 512 + off + n],
                        lhsT=kt_sb[v["hp"]][hi * 64:(hi + 1) * 64,
                                            kc * 128:(kc + 1) * 128],
                        rhs=qt_sb[v["hp"]][hi * 64:(hi + 1) * 64, q0: q0 + n],
                        start=(si == 0), stop=False)
            last = len(v["subs"]) - 1
            for si, (kc, q0, n, off) in enumerate(v["subs"]):
                seb = q0 - 128 * kc + 512
                for hi in range(2):
                    nc.tensor.matmul(
                        sc[:, hi * 512 + off: hi * 512 + off + n],
                        lhsT=idn_sb[:],
                        rhs=eb_sb[:, seb:seb + n],
                        start=False, stop=(si == last))

        def expp(u):
            # exp runs one pipeline step behind scores; PV two steps behind,
            # so the PE fills the ACT latency with PV work instead of
            # stalling right after the next unit's score matmuls
            v = units[u]
            sc = state.pop(u)
            wg = v["subs"][-1][3] + v["subs"][-1][2]  # group width
            et = ets.tile([128, 1024], bf16, tag="et", name=f"et{u}")
            et2 = et.rearrange("p (r w) -> p r w", r=2)
            sc3 = sc.rearrange("p (r w) -> p r w", r=2)
            nc.scalar.activation(et2[:, :, 0:wg], sc3[:, :, 0:wg],
                                 AF.Exp, scale=0.125)
            state[("et", u)] = et2

        def pvpart(u):
            v = units[u]
            qt, hp = v["qt"], v["hp"]
            et2 = state.pop(("et", u))
            if v["first"]:
                state[("po", qt, hp)] = psum.tile(
                    [65, 1024], f32, tag="pa", bufs=2, name=f"po{qt}_{hp}")
            po = state[("po", qt, hp)]
            last = len(v["subs"]) - 1
            for si, (kc, q0, n, off) in enumerate(v["subs"]):
                qoff = q0 - 512 * qt
                for hi in range(2):
                    h = 2 * hp + hi
                    vsl = vb_sb[:, kc * VW + h * 65: kc * VW + h * 65 + 65]
                    nc.tensor.matmul(
                        po[0:65, hi * 512 + qoff: hi * 512 + qoff + n],
                        lhsT=vsl,
                        rhs=et2[:, hi, off:off + n],
                        start=(v["first"] and si == 0),
                        stop=(v["last"] and si == last))
            if not v["last"]:
                return
            # normalize: denominators sit in po row 64 (both heads); stage to
            # partition 0 first (the custom-DVE recip can't cross 32-strips)
            po = state.pop(("po", qt, hp))
            if qt == NQT - 1 and hp == HP - 1:
                # the final pair's chain is the only one nothing overlaps:
                # split it into two parallel per-half chains to cut latency
                for hi in range(2):
                    cs = slice(hi * 512, hi * 512 + 512)
                    ss = rbp.tile([1, 512], f32, tag=f"ssh{hi}", name=f"ssh{hi}")
                    if hi == 0:
                        nc.scalar.copy(ss[:], po[64:65, cs])
                    else:
                        nc.vector.tensor_copy(ss[:], po[64:65, cs])
                    rf = rbp.tile([1, 512], f32, tag=f"rfh{hi}", name=f"rfh{hi}")
                    nc.vector.reciprocal_approx_fast(rf[:], ss[:])
                    rb = rbp.tile([64, 512], f32, tag=f"rbh{hi}", name=f"rbh{hi}")
                    nc.gpsimd.partition_broadcast(rb[:], rf[:])
                    sl = (slice(hi * 64, (hi + 1) * 64),
                          slice(qt * 512, (qt + 1) * 512))
                    nc.vector.tensor_mul(ots_sb[hp][sl], po[0:64, cs], rb[:])
                return
            s_st = rbp.tile([1, 1024], f32, tag="ss", name=f"ss{qt}_{hp}")
            if hp % 2 == 0:
                nc.scalar.copy(s_st[:], po[64:65, :])
            else:
                nc.vector.tensor_copy(s_st[:], po[64:65, :])
            r_f = rbp.tile([1, 1024], f32, tag="rf", name=f"rf{qt}_{hp}")
            nc.vector.reciprocal_approx_fast(r_f[:], s_st[:])
            rbb = rbp.tile([64, 1024], f32, tag="rbb", name=f"rbb{qt}_{hp}")
            nc.gpsimd.partition_broadcast(rbb[:], r_f[:])
            for hi in range(2):
                sl = (slice(hi * 64, (hi + 1) * 64),
                      slice(qt * 512, (qt + 1) * 512))
                nc.vector.tensor_mul(
                    ots_sb[hp][sl],
                    po[0:64, hi * 512: hi * 512 + 512],
                    rbb[:, hi * 512: hi * 512 + 512])

        U = len(units)
        scores(0)
        scores(1)
        expp(0)
        for u in range(2, U):
            scores(u)
            expp(u - 1)
            pvpart(u - 2)
            for qc in after_unit.get(u - 2, ()):
                outproj_qc(qc)
        expp(U - 1)
        for u in (U - 2, U - 1):
            pvpart(u)
            for qc in after_unit.get(u, ()):
                outproj_qc(qc)

    nc.compile()
    return nc


def host_inputs(inputs, cfg=FULL):
    """Build the 8 per-core input maps + the host-side combine constant."""
    L, C, DL, NHL = cfg.L, cfg.C, cfg.DL, cfg.NHL
    q = np.asarray(inputs["queries"], np.float32)
    k = np.asarray(inputs["keys"], np.float32)
    v = np.asarray(inputs["values"], np.float32)
    Wq = np.asarray(inputs["Wq"], np.float32)
    Wk = np.asarray(inputs["Wk"], np.float32)
    Wv = np.asarray(inputs["Wv"], np.float32)
    Wo = np.asarray(inputs["Wo"], np.float32)
    bq = np.asarray(inputs["bq"], np.float32)
    bv = np.asarray(inputs["bv"], np.float32)
    bo = np.asarray(inputs["bo"], np.float32)
    B = q.shape[0]

    bo_eff = (bo.astype(np.float64) + Wo.astype(np.float64) @ bv.astype(np.float64)
              ).astype(np.float32)

    p = np.arange(128, dtype=np.float64)[:, None]
    c = np.arange(cfg.EBW, dtype=np.float64)[None, :]
    # additive log-bias, pre-divided by the 0.125 softmax scale:
    # exp(0.125*(s + eb)) = exp(s/8 - 0.1|k-q|)
    eb = (-0.8 * np.abs(p - c + 512)).astype(BF16)
    idn = np.eye(128, dtype=BF16)

    xT = {}
    for b in range(B):
        xT[b] = (np.ascontiguousarray(q[b].T).astype(BF16),
                 np.ascontiguousarray(k[b].T).astype(BF16),
                 np.ascontiguousarray(v[b].T).astype(BF16))

    in_maps = []
    for core in range(N_CORES):
        b, hg = core // 2, core % 2
        sl = slice(hg * DL, (hg + 1) * DL)
        in_maps.append({
            "xq": xT[b][0], "xk": xT[b][1], "xv": xT[b][2],
            "wq": np.ascontiguousarray(Wq.T[:, sl]).astype(BF16),
            "wk": np.ascontiguousarray(Wk.T[:, sl]).astype(BF16),
            "wv": np.ascontiguousarray(Wv.T[:, sl]).astype(BF16),
            "wo": np.ascontiguousarray(Wo.T[sl, :]).astype(BF16),
            "bq": np.ascontiguousarray(bq[sl][:, None]),
            "eb": eb, "idn": idn,
        })
    return in_maps, bo_eff


_CACHED = {}


def _wait_devices_healthy(timeout_s=420):
    import time
    import jax
    import jax.numpy as jnp
    t0 = time.time()
    last = None
    while time.time() - t0 < timeout_s:
        try:
            for d in jax.devices():
                x = jax.device_put(np.ones((8, 8), np.float32), d)
                jnp.sum(x).block_until_ready()
            return
        except Exception as e:  # wedged worker recycles within a few minutes
            last = e
            time.sleep(15)
    raise RuntimeError(f"NeuronCores unhealthy after {timeout_s}s: {last}")


def kernel(**inputs):
    from concourse.bass_utils import run_bass_kernel_spmd

    cfg = FULL
    if "nc" not in _CACHED:
        _CACHED["nc"] = build_program(cfg)
    nc = _CACHED["nc"]

    in_maps, bo_eff = host_inputs(inputs, cfg)
    _wait_devices_healthy()
    try:
        res = run_bass_kernel_spmd(nc, in_maps, core_ids=list(range(N_CORES)))
    except Exception:
        _wait_devices_healthy()
        res = run_bass_kernel_spmd(nc, in_maps, core_ids=list(range(N_CORES)))
    B = np.asarray(inputs["queries"]).shape[0]
    out = np.zeros((B, cfg.L, cfg.C), np.float32)
    for b in range(B):
        out[b] = (res.results[2 * b]["out"].astype(np.float32)
                  + res.results[2 * b + 1]["out"].astype(np.float32)
                  + bo_eff[None, :])
    return out


# revision 44
# speedup vs baseline: 1.2186x; 1.0076x over previous
"""Trainium2 Bass kernel for nn_AutoCorrelation (multi-head attention with a
distance decay bias), SPMD across 8 NeuronCores.

Sharding: core = (batch b, head-group hg) with b in 0..3, hg in 0..1.
Each core computes, for its batch and its 8 heads: QKV projections
(column-sharded weights), distance-banded attention (the -0.1*|i-j| bias makes
weights beyond |i-j|~96 numerically zero), and a row-sharded output
projection. The host sums the two half partial outputs per batch and adds the
effective output bias.

Math notes:
 - bk drops out entirely (softmax row-shift invariance); bv is folded into the
   host-side output bias: bo_eff = bo + Wo @ bv.
 - scores are built transposed St[k, q]; a ones-column appended to V yields the
   softmax denominators in the same matmul (PSUM row 64).
 - the bias exp(-0.1|k-q|) is a Toeplitz multiply from a precomputed master.

Structure (v2):
 - warmup matmuls at t=0 keep the PE HAM un-throttled while input DMA streams;
   weight DMAs issue from gpsimd in parallel with x loads on sync.
 - Phase B runs as 4 q-tile passes; per pass all 8 heads are processed as 4
   pairs with concurrent K=64 score matmuls (tile_position row groups), one
   merged exp ACTIVATE per pair, and the output projection + out DMA run
   inside the pass so HBM writes spread across the kernel.
"""

import math
from contextlib import ExitStack

import numpy as np
import ml_dtypes

BF16 = ml_dtypes.bfloat16

N_CORES = 8


class Cfg:
    def __init__(self, L=2048, C=1024, NHL=8, DK=64, W=96):
        self.L, self.C, self.NHL, self.DK, self.W = L, C, NHL, DK, W
        self.DL = NHL * DK               # local head dims
        self.SPAN = 128 + 2 * W          # k-chunk q-span
        self.KC = L // 128               # k chunks
        self.NQT = L // 512              # q tiles (512)
        self.CC = C // 128               # contraction chunks
        self.LT = L // 512               # l tiles
        self.HP = NHL // 2               # head pairs
        self.VW = NHL * 65               # padded V width
        self.EBW = self.SPAN + 576       # EB master width (seb range + span)
        assert self.SPAN <= self.L

    def qs_of(self, kc):
        return min(max(128 * kc - self.W, 0), self.L - self.SPAN)

    def pieces_of(self, qt):
        """(kc, q0, N) pieces whose span intersects q-tile qt."""
        lo, hi = 512 * qt, 512 * qt + 512
        out = []
        for kc in range(self.KC):
            qs = self.qs_of(kc)
            q0, q1 = max(qs, lo), min(qs + self.SPAN, hi)
            if q1 > q0:
                out.append((kc, q0, q1 - q0))
        return out


FULL = Cfg(W=80)


def build_program(cfg=FULL, debug=False):
    import concourse.bass as bass
    import concourse.tile as tile
    from concourse import bacc, mybir

    f32 = mybir.dt.float32
    bf16 = mybir.dt.bfloat16
    AF = mybir.ActivationFunctionType

    L, C, NHL, DL = cfg.L, cfg.C, cfg.NHL, cfg.DL
    KC, NQT, CC, LT, HP, VW = cfg.KC, cfg.NQT, cfg.CC, cfg.LT, cfg.HP, cfg.VW

    nc = bacc.Bacc("TRN2", target_bir_lowering=False, debug=debug,
                   num_devices=N_CORES)

    xq = nc.dram_tensor("xq", [C, L], bf16, kind="ExternalInput").ap()
    xk = nc.dram_tensor("xk", [C, L], bf16, kind="ExternalInput").ap()
    xv = nc.dram_tensor("xv", [C, L], bf16, kind="ExternalInput").ap()
    wq = nc.dram_tensor("wq", [C, DL], bf16, kind="ExternalInput").ap()
    wk = nc.dram_tensor("wk", [C, DL], bf16, kind="ExternalInput").ap()
    wv = nc.dram_tensor("wv", [C, DL], bf16, kind="ExternalInput").ap()
    wo = nc.dram_tensor("wo", [DL, C], bf16, kind="ExternalInput").ap()
    bqd = nc.dram_tensor("bq", [DL, 1], f32, kind="ExternalInput").ap()
    ebd = nc.dram_tensor("eb", [128, cfg.EBW], bf16, kind="ExternalInput").ap()
    idnd = nc.dram_tensor("idn", [128, 128], bf16, kind="ExternalInput").ap()
    out = nc.dram_tensor("out", [L, C], bf16, kind="ExternalOutput").ap()

    with tile.TileContext(nc) as tc, ExitStack() as ctx:
        const = ctx.enter_context(tc.tile_pool(name="const", bufs=1))
        big = ctx.enter_context(tc.tile_pool(name="big", bufs=1))
        xs = ctx.enter_context(tc.tile_pool(name="xs", bufs=3))
        ets = ctx.enter_context(tc.tile_pool(name="ets", bufs=4))
        rbp = ctx.enter_context(tc.tile_pool(name="rbp", bufs=2))
        ostage = ctx.enter_context(tc.tile_pool(name="ostage", bufs=3))
        psum = ctx.enter_context(tc.tile_pool(name="psum", bufs=1, space="PSUM"))

        # ---- PE warmup: keep HAM un-throttled while input DMA streams ----
        dummy = const.tile([128, 512], bf16)
        nc.vector.memset(dummy[:], 0.0)
        # dummy exp pulls the ACT table load off the critical path
        dummy_act = const.tile([1, 8], f32)
        nc.scalar.activation(dummy_act[:], dummy[0:1, 0:8], AF.Exp, scale=0.125)
        warm_ps = psum.tile([128, 1024], f32, tag="pa", bufs=2, name="warm")
        for i in range(28):
            nc.tensor.matmul(warm_ps[:, 0:512], lhsT=dummy[:, 0:128],
                             rhs=dummy[:], start=True, stop=True)
        warm_sink = const.tile([128, 512], bf16)
        nc.vector.tensor_copy(warm_sink[:], warm_ps[:, 0:512])

        # ---- resident constants (weight DMAs on gpsimd, x loads on sync) ----
        wq_sb = const.tile([128, CC * DL], bf16)
        wk_sb = const.tile([128, CC * DL], bf16)
        wv_sb = const.tile([128, CC * DL], bf16)
        wo_sb = const.tile([128, HP * C], bf16)
        bq_sb = const.tile([128, HP], f32)
        # two 128-row chunks per descriptor: halves the ~600ns-per-dma_start
        # issue cost that was starving the early projection matmuls
        def load_w2(dst_sb, src):
            for c2 in range(CC // 2):
                nc.gpsimd.dma_start(
                    dst_sb[:, c2 * 2 * DL:(c2 + 1) * 2 * DL].rearrange(
                        "p (r w) -> p r w", r=2),
                    src[c2 * 256:(c2 + 1) * 256, :].rearrange(
                        "(r p) w -> p r w", p=128))

        load_w2(wq_sb, wq)
        # bq after wq (it gates the first bias ACT at ~20us, wq gates the
        # first matmuls at ~8us)
        for hp in range(HP):
            nc.gpsimd.dma_start(bq_sb[:, hp:hp + 1], bqd[hp * 128:(hp + 1) * 128, :])
        load_w2(wk_sb, wk)
        load_w2(wv_sb, wv)
        eb_sb = const.tile([128, cfg.EBW], bf16)
        nc.gpsimd.dma_start(eb_sb[:], ebd[:])
        idn_sb = const.tile([128, 128], bf16)
        nc.gpsimd.dma_start(idn_sb[:], idnd[:])
        for hp in range(HP):
            nc.gpsimd.dma_start(wo_sb[:, hp * C:(hp + 1) * C], wo[hp * 128:(hp + 1) * 128, :])

        # ---- resident activations ----
        qt_sb = [big.tile([128, L], bf16, name=f"qt{hp}") for hp in range(HP)]
        kt_sb = [big.tile([128, L], bf16, name=f"kt{hp}") for hp in range(HP)]
        vb_sb = big.tile([128, KC * VW], bf16)
        ots_sb = [big.tile([128, L], bf16, name=f"ots{hp}") for hp in range(HP)]

        # ones columns of the [V_h | ones] layout, written once on DVE
        for kcg in range(KC):
            vbk = vb_sb[:, kcg * VW:(kcg + 1) * VW].rearrange(
                "p (h w) -> p h w", w=65)
            nc.vector.memset(vbk[:, :, 64:65], 1.0)

        # ================= Phase A: projections =================
        # Alternate projection PSUM groups across both ring tags ("pa"/"sc")
        # for an effective 4-deep pipeline: the drain of group i no longer
        # gates the matmuls of group i+2.
        alt = [0]

        def proj_ps(shape, name):
            alt[0] += 1
            tag = "pa" if alt[0] % 2 else "sc"
            return psum.tile(shape, f32, tag=tag, bufs=2, name=name)

        for lt in range(LT):
            for which, xdram in (("q", xq), ("k", xk), ("v", xv)):
                x_sb = xs.tile([128, CC * 512], bf16, tag="xs",
                               name=f"x_{which}{lt}")
                for c2 in range(CC // 2):
                    nc.sync.dma_start(
                        x_sb[:, c2 * 1024:(c2 + 1) * 1024].rearrange(
                            "p (r w) -> p r w", r=2),
                        xdram[c2 * 256:(c2 + 1) * 256,
                              lt * 512:(lt + 1) * 512].rearrange(
                            "(r p) w -> p r w", p=128))
                if which in ("q", "k"):
                    w_sb = wq_sb if which == "q" else wk_sb
                    t_sb = qt_sb if which == "q" else kt_sb
                    for hp in range(HP):
                        ps = proj_ps([128, 512], f"psp_{which}{lt}_{hp}")
                        for c in range(CC):
                            nc.tensor.matmul(
                                ps[:],
                                lhsT=w_sb[:, c * DL + hp * 128: c * DL + hp * 128 + 128],
                                rhs=x_sb[:, c * 512:(c + 1) * 512],
                                start=(c == 0), stop=(c == CC - 1))
                        dst = t_sb[hp][:, lt * 512:(lt + 1) * 512]
                        if which == "q":
                            nc.scalar.activation(dst, ps[:], AF.Identity,
                                                 bias=bq_sb[:, hp:hp + 1], scale=1.0)
                        else:
                            nc.vector.tensor_copy(dst, ps[:])
                else:
                    for sub in range(4):
                        kcg = lt * 4 + sub
                        ps = proj_ps([128, DL], f"psp_v{kcg}")
                        for c in range(CC):
                            lhsT = x_sb[:, c * 512 + sub * 128: c * 512 + sub * 128 + 128]
                            nc.tensor.matmul(
                                ps[:], lhsT=lhsT,
                                rhs=wv_sb[:, c * DL:(c + 1) * DL],
                                start=(c == 0), stop=(c == CC - 1))
                        vbk = vb_sb[:, kcg * VW:(kcg + 1) * VW].rearrange(
                            "p (h w) -> p h w", w=65)
                        nc.vector.tensor_copy(
                            vbk[:, :, 0:64],
                            ps.rearrange("p (h w) -> p h w", w=64))

        # ================= Phase B: banded attention, q-tile passes =========
        # Flat software pipeline over all (qt, head-pair, kc-piece) units with
        # one-piece lookahead crossing pair and qt boundaries.  PSUM: "sc"
        # ring 2x2 banks (scores + outproj pf), "pa" ring 2x2 banks (Phase A
        # proj + po accumulators) = 8 banks.  The distance bias -0.8|k-q| is
        # added into the score PSUM by an identity matmul, so exp(0.125*x)
        # yields the biased weights directly (no separate DVE multiply).

        def outproj_qc(qc):
            st = ostage.tile([128, C], bf16, tag="fo", name=f"fo{qc}")
            pf = psum.tile([128, 1024], f32, tag="sc", bufs=2, name=f"pf{qc}")
            for mi, mo in enumerate((0, 512)):
                for hp2 in range(HP):
                    nc.tensor.matmul(
                        pf[:, mo:mo + 512],
                        lhsT=ots_sb[hp2][:, qc * 128:(qc + 1) * 128],
                        rhs=wo_sb[:, hp2 * C + mo: hp2 * C + mo + 512],
                        start=(hp2 == 0), stop=(hp2 == HP - 1))
            if qc % 2 == 0:
                nc.scalar.copy(st[:], pf[:])
            else:
                nc.vector.tensor_copy(st[:], pf[:])
            nc.gpsimd.dma_start(out[qc * 128:(qc + 1) * 128, :], st[:])

        units = []
        qt_start = {}
        for qt in range(NQT):
            pieces = cfg.pieces_of(qt)
            # greedy-pack consecutive kc-pieces into groups of total width
            # <=512 so each group needs one PSUM bank per head and one exp
            groups = []
            for kc, q0, n in pieces:
                if groups and groups[-1][-1][3] + groups[-1][-1][2] + n <= 512:
                    off = groups[-1][-1][3] + groups[-1][-1][2]
                    groups[-1].append((kc, q0, n, off))
                else:
                    groups.append([(kc, q0, n, 0)])
            qt_start[qt] = len(units)
            for hp in range(HP):
                for idx, subs in enumerate(groups):
                    units.append(dict(
                        qt=qt, hp=hp, subs=subs,
                        first=(idx == 0), last=(idx == len(groups) - 1)))
        # outproj(qt) is delayed into qt+1's unit stream (one qc per ~2
        # units) so the normalization chain overlaps flowing attention work
        # and the outproj matmuls act as PE filler
        after_unit = {}
        for qt in range(NQT):
            for j in range(4):
                qc = 4 * qt + j
                if qt + 1 < NQT:
                    key = qt_start[qt + 1] + 4 * j + 2
                else:
                    key = len(units) - 1
                after_unit.setdefault(key, []).append(qc)

        state = {}

        def scores(u):
            v = units[u]
            sc = psum.tile([128, 1024], f32, tag="sc", bufs=2, name=f"sc{u}")
            state[u] = sc
            for si, (kc, q0, n, off) in enumerate(v["subs"]):
                for hi in range(2):
                    nc.tensor.matmul(
                        sc[:, hi * 512 + off: hi * 512 + off + n],
                        lhsT=kt_sb[v["hp"]][hi * 64:(hi + 1) * 64,
                                            kc * 128:(kc + 1) * 128],
                        rhs=qt_sb[v["hp"]][hi * 64:(hi + 1) * 64, q0: q0 + n],
                        start=(si == 0), stop=False)
            last = len(v["subs"]) - 1
            for si, (kc, q0, n, off) in enumerate(v["subs"]):
                seb = q0 - 128 * kc + 512
                for hi in range(2):
                    nc.tensor.matmul(
                        sc[:, hi * 512 + off: hi * 512 + off + n],
                        lhsT=idn_sb[:],
                        rhs=eb_sb[:, seb:seb + n],
                        start=False, stop=(si == last))

        def expp(u):
            # exp runs one pipeline step behind scores; PV two steps behind,
            # so the PE fills the ACT latency with PV work instead of
            # stalling right after the next unit's score matmuls
            v = units[u]
            sc = state.pop(u)
            wg = v["subs"][-1][3] + v["subs"][-1][2]  # group width
            et = ets.tile([128, 1024], bf16, tag="et", name=f"et{u}")
            et2 = et.rearrange("p (r w) -> p r w", r=2)
            sc3 = sc.rearrange("p (r w) -> p r w", r=2)
            nc.scalar.activation(et2[:, :, 0:wg], sc3[:, :, 0:wg],
                                 AF.Exp, scale=0.125)
            state[("et", u)] = et2

        def pvpart(u):
            v = units[u]
            qt, hp = v["qt"], v["hp"]
            et2 = state.pop(("et", u))
            if v["first"]:
                state[("po", qt, hp)] = psum.tile(
                    [65, 1024], f32, tag="pa", bufs=2, name=f"po{qt}_{hp}")
            po = state[("po", qt, hp)]
            last = len(v["subs"]) - 1
            for si, (kc, q0, n, off) in enumerate(v["subs"]):
                qoff = q0 - 512 * qt
                for hi in range(2):
                    h = 2 * hp + hi
                    vsl = vb_sb[:, kc * VW + h * 65: kc * VW + h * 65 + 65]
                    nc.tensor.matmul(
                        po[0:65, hi * 512 + qoff: hi * 512 + qoff + n],
                        lhsT=vsl,
                        rhs=et2[:, hi, off:off + n],
                        start=(v["first"] and si == 0),
                        stop=(v["last"] and si == last))
            if not v["last"]:
                return
            # normalize: denominators sit in po row 64 (both heads); stage to
            # partition 0 first (the custom-DVE recip can't cross 32-strips)
            po = state.pop(("po", qt, hp))
            if qt == NQT - 1 and hp == HP - 1:
                # the final pair's chain is the only one nothing overlaps:
                # split it into two parallel per-half chains to cut latency
                for hi in range(2):
                    cs = slice(hi * 512, hi * 512 + 512)
                    ss = rbp.tile([1, 512], f32, tag=f"ssh{hi}", name=f"ssh{hi}")
                    if hi == 0:
                        nc.scalar.copy(ss[:], po[64:65, cs])
                    else:
                        nc.vector.tensor_copy(ss[:], po[64:65, cs])
                    rf = rbp.tile([1, 512], f32, tag=f"rfh{hi}", name=f"rfh{hi}")
                    nc.vector.reciprocal_approx_fast(rf[:], ss[:])
                    rb = rbp.tile([64, 512], f32, tag=f"rbh{hi}", name=f"rbh{hi}")
                    nc.gpsimd.partition_broadcast(rb[:], rf[:])
                    sl = (slice(hi * 64, (hi + 1) * 64),
                          slice(qt * 512, (qt + 1) * 512))
                    nc.vector.tensor_mul(ots_sb[hp][sl], po[0:64, cs], rb[:])
                return
            s_st = rbp.tile([1, 1024], f32, tag="ss", name=f"ss{qt}_{hp}")
            if hp % 2 == 0:
                nc.scalar.copy(s_st[:], po[64:65, :])
            else:
                nc.vector.tensor_copy(s_st[:], po[64:65, :])
            r_f = rbp.tile([1, 1024], f32, tag="rf", name=f"rf{qt}_{hp}")
            nc.vector.reciprocal_approx_fast(r_f[:], s_st[:])
            rbb = rbp.tile([64, 1024], f32, tag="rbb", name=f"rbb{qt}_{hp}")
            nc.gpsimd.partition_broadcast(rbb[:], r_f[:])
            for hi in range(2):
                sl = (slice(hi * 64, (hi + 1) * 64),
                      slice(qt * 512, (qt + 1) * 512))
                nc.vector.tensor_mul(
                    ots_sb[hp][sl],
                    po[0:64, hi * 512: hi * 512 + 512],
                    rbb[:, hi * 512: hi * 512 + 512])

        U = len(units)
        scores(0)
        scores(1)
        expp(0)
        for u in range(2, U):
            scores(u)
            expp(u - 1)
            pvpart(u - 2)
            for qc in after_unit.get(u - 2, ()):
                outproj_qc(qc)
        expp(U - 1)
        for u in (U - 2, U - 1):
            pvpart(u)
            for qc in after_unit.get(u, ()):
                outproj_qc(qc)

    nc.compile()
    return nc


def host_inputs(inputs, cfg=FULL):
    """Build the 8 per-core input maps + the host-side combine constant."""
    L, C, DL, NHL = cfg.L, cfg.C, cfg.DL, cfg.NHL
    q = np.asarray(inputs["queries"], np.float32)
    k = np.asarray(inputs["keys"], np.float32)
    v = np.asarray(inputs["values"], np.float32)
    Wq = np.asarray(inputs["Wq"], np.float32)
    Wk = np.asarray(inputs["Wk"], np.float32)
    Wv = np.asarray(inputs["Wv"], np.float32)
    Wo = np.asarray(inputs["Wo"], np.float32)
    bq = np.asarray(inputs["bq"], np.float32)
    bv = np.asarray(inputs["bv"], np.float32)
    bo = np.asarray(inputs["bo"], np.float32)
    B = q.shape[0]

    bo_eff = (bo.astype(np.float64) + Wo.astype(np.float64) @ bv.astype(np.float64)
              ).astype(np.float32)

    p = np.arange(128, dtype=np.float64)[:, None]
    c = np.arange(cfg.EBW, dtype=np.float64)[None, :]
    # additive log-bias, pre-divided by the 0.125 softmax scale:
    # exp(0.125*(s + eb)) = exp(s/8 - 0.1|k-q|)
    eb = (-0.8 * np.abs(p - c + 512)).astype(BF16)
    idn = np.eye(128, dtype=BF16)

    xT = {}
    for b in range(B):
        xT[b] = (np.ascontiguousarray(q[b].T).astype(BF16),
                 np.ascontiguousarray(k[b].T).astype(BF16),
                 np.ascontiguousarray(v[b].T).astype(BF16))

    in_maps = []
    for core in range(N_CORES):
        b, hg = core // 2, core % 2
        sl = slice(hg * DL, (hg + 1) * DL)
        in_maps.append({
            "xq": xT[b][0], "xk": xT[b][1], "xv": xT[b][2],
            "wq": np.ascontiguousarray(Wq.T[:, sl]).astype(BF16),
            "wk": np.ascontiguousarray(Wk.T[:, sl]).astype(BF16),
            "wv": np.ascontiguousarray(Wv.T[:, sl]).astype(BF16),
            "wo": np.ascontiguousarray(Wo.T[sl, :]).astype(BF16),
            "bq": np.ascontiguousarray(bq[sl][:, None]),
            "eb": eb, "idn": idn,
        })
    return in_maps, bo_eff


_CACHED = {}


def _wait_devices_healthy(timeout_s=420):
    import time
    import jax
    import jax.numpy as jnp
    t0 = time.time()
    last = None
    while time.time() - t0 < timeout_s:
        try:
            for d in jax.devices():
                x = jax.device_put(np.ones((8, 8), np.float32), d)
                jnp.sum(x).block_until_ready()
            return
        except Exception as e:  # wedged worker recycles within a few minutes
            last = e
            time.sleep(15)
    raise RuntimeError(f"NeuronCores unhealthy after {timeout_s}s: {last}")


def kernel(**inputs):
    from concourse.bass_utils import run_bass_kernel_spmd

    cfg = FULL
    if "nc" not in _CACHED:
        _CACHED["nc"] = build_program(cfg)
    nc = _CACHED["nc"]

    in_maps, bo_eff = host_inputs(inputs, cfg)
    _wait_devices_healthy()
    try:
        res = run_bass_kernel_spmd(nc, in_maps, core_ids=list(range(N_CORES)))
    except Exception:
        _wait_devices_healthy()
        res = run_bass_kernel_spmd(nc, in_maps, core_ids=list(range(N_CORES)))
    B = np.asarray(inputs["queries"]).shape[0]
    out = np.zeros((B, cfg.L, cfg.C), np.float32)
    for b in range(B):
        out[b] = (res.results[2 * b]["out"].astype(np.float32)
                  + res.results[2 * b + 1]["out"].astype(np.float32)
                  + bo_eff[None, :])
    return out
